# revision 1
# baseline (speedup 1.0000x reference)
"""Trainium2 Bass kernel for nn_Block_74363063763569 (BEiT-style transformer block).

Data-parallel over batch across 8 NeuronCores (8 elems/core), zero collectives.
Self-contained: builds, compiles (cached) and runs the Bass kernel via
run_bass_kernel_spmd on cores 0-7.
"""
import sys, json
sys.path.insert(0, "/opt/trn_rl_repo")
import numpy as np


def _legalize_waits(bir_bytes, max_waits=1):
    """This container's walrus rejects >1 sync wait per instruction; split
    extras into preceding single-wait EventSemaphore instructions."""
    j = json.loads(bir_bytes)
    for f in j["functions"]:
        for b in f["blocks"]:
            out = []
            for inst in b["instructions"]:
                si = inst.get("sync_info")
                waits = si.get("on_wait", []) if si else []
                if len(waits) > max_waits:
                    keep, extra = waits[:max_waits], waits[max_waits:]
                    for k, w in enumerate(extra):
                        out.append({"debug": inst.get("debug", 0), "engine": inst["engine"],
                                    "ins": [], "name": f"{inst['name']}_w{k}",
                                    "opcode": "EventSemaphore", "outs": [],
                                    "sync_info": {"on_update": [], "on_wait": [w]}})
                    si["on_wait"] = keep
                out.append(inst)
            b["instructions"] = out
    return json.dumps(j).encode()


"""Bass/Tile kernel builder for the BEiT-style transformer block.

Strategy (per core, data-parallel over batch):
- 8 batch elements per core, processed as 4 pairs (token axis packed to 394).
- Residual stream kept in NORMAL layout [tokens, features] fp32.
- Matmul activations in TRANSPOSED layout [features, tokens] bf16
  (produced via PE transposes of the LN outputs).
- LN affine folded into qkv/fc1 weights; gamma1/gamma2 folded into
  proj/fc2 weights; attention scale folded into q weights (host side).
- Attention: logits computed transposed [t_k, t_q]; softmax without
  max-subtraction (logits are provably small); denominator via a ones
  column appended to V; per-column normalization via GPSIMD
  partition_broadcast of the reciprocal row.
"""

import numpy as np

import concourse.bass as bass
import concourse.tile as tile
import concourse.mybir as mybir
from concourse.masks import make_identity

FP32 = mybir.dt.float32
BF16 = mybir.dt.bfloat16

B = 64
N = 197
C = 768
H = 12
D = 64
HID = 3072
NCORES = 8
BPC = B // NCORES          # 8 batch elems per core
NPAIRS_FULL = BPC // 2     # 4
KT = C // 128              # 6 k-tiles of 128 over features
KT2 = HID // 128           # 24 k-tiles over hidden
LN_EPS = 1e-5

# token tiling: 197 = 128 + 69
T_TILES = [(0, 128), (128, 69)]
# output chunks over feature dim 768 = 512 + 256
C_CHUNKS = [(0, 512), (512, 256)]

AL = mybir.AluOpType
AF = mybir.ActivationFunctionType


def build_nc(npairs=NPAIRS_FULL):
    nb = 2 * npairs
    nc = bass.Bass()

    x_d = nc.dram_tensor("x", [nb, N, C], FP32, kind="ExternalInput")
    qkvT_d = nc.dram_tensor("qkvT", [C, 3 * C], BF16, kind="ExternalInput")
    projT_d = nc.dram_tensor("projT", [C, C], BF16, kind="ExternalInput")
    fc1T_d = nc.dram_tensor("fc1T", [C, HID], BF16, kind="ExternalInput")
    fc2T_d = nc.dram_tensor("fc2T", [HID, C], BF16, kind="ExternalInput")
    rpb0_d = nc.dram_tensor("rpb0", [128, H, N], BF16, kind="ExternalInput")
    rpb1_d = nc.dram_tensor("rpb1", [69, H, N], BF16, kind="ExternalInput")
    qb_d = nc.dram_tensor("qb", [C], FP32, kind="ExternalInput")
    kb_d = nc.dram_tensor("kb", [C], FP32, kind="ExternalInput")
    fc1b_d = nc.dram_tensor("fc1b", [HID], FP32, kind="ExternalInput")
    vb_d = nc.dram_tensor("vbrow", [C], BF16, kind="ExternalInput")
    pb_d = nc.dram_tensor("pbrow", [C], BF16, kind="ExternalInput")
    f2b_d = nc.dram_tensor("f2brow", [C], BF16, kind="ExternalInput")
    y_d = nc.dram_tensor("y", [nb, N, C], FP32, kind="ExternalOutput")

    with tile.TileContext(nc) as tc:
        with (
            tc.tile_pool(name="singles", bufs=1) as singles,
            tc.tile_pool(name="resid", bufs=1) as resid,     # x0/x1/out fp32
            tc.tile_pool(name="b394", bufs=1) as b394,      # bf16 [128,394] transient
            tc.tile_pool(name="xn", bufs=3) as xnp,
            tc.tile_pool(name="vpool", bufs=4) as vpool,
            tc.tile_pool(name="expp", bufs=4) as expp,
            tc.tile_pool(name="dpool", bufs=2) as dpool,
            tc.tile_pool(name="small", bufs=8) as small,
            tc.tile_pool(name="ps_tr", bufs=2, space="PSUM") as ps_tr,
            tc.tile_pool(name="ps_mm", bufs=2, space="PSUM") as ps_mm,
            tc.tile_pool(name="ps_at", bufs=4, space="PSUM") as ps_at,
        ):
            # ---- persistent weights / constants ----
            qkvT = [singles.tile([128, 3 * C], BF16, tag=f"qkvT{k}", name=f"qkvT{k}") for k in range(KT)]
            projT = [singles.tile([128, C], BF16, tag=f"projT{k}", name=f"projT{k}") for k in range(KT)]
            fc1T = [singles.tile([128, HID], BF16, tag=f"fc1T{k}", name=f"fc1T{k}") for k in range(KT)]
            fc2T = [singles.tile([128, C], BF16, tag=f"fc2T{k}", name=f"fc2T{k}") for k in range(KT2)]
            rpb0 = singles.tile([128, H, N], BF16, tag="rpb0")
            rpb1 = singles.tile([69, H, N], BF16, tag="rpb1")
            qb_sb = singles.tile([128, KT], FP32, tag="qb")
            kb_sb = singles.tile([128, KT], FP32, tag="kb")
            fc1b_sb = singles.tile([128, KT2], FP32, tag="fc1b")
            vb_sb = singles.tile([1, C], BF16, tag="vb")
            pb_sb = singles.tile([1, C], BF16, tag="pb")
            f2b_sb = singles.tile([1, C], BF16, tag="f2b")
            ident = singles.tile([128, 128], BF16, tag="ident")
            ones_row = singles.tile([1, 128], BF16, tag="ones")
            ones_f32 = singles.tile([1, 128], FP32, tag="ones32")
            eps_sb = singles.tile([128, 1], FP32, tag="eps")

            for k in range(KT):
                nc.sync.dma_start(qkvT[k][:], qkvT_d[k * 128:(k + 1) * 128, :])
            for k in range(KT):
                nc.sync.dma_start(projT[k][:], projT_d[k * 128:(k + 1) * 128, :])
            for k in range(KT):
                nc.sync.dma_start(fc1T[k][:], fc1T_d[k * 128:(k + 1) * 128, :])
            for k in range(KT2):
                nc.sync.dma_start(fc2T[k][:], fc2T_d[k * 128:(k + 1) * 128, :])
            nc.sync.dma_start(rpb0[:], rpb0_d[:])
            nc.sync.dma_start(rpb1[:], rpb1_d[:])
            nc.sync.dma_start(qb_sb[:], qb_d[:].rearrange("(k p) -> p k", p=128))
            nc.sync.dma_start(kb_sb[:], kb_d[:].rearrange("(k p) -> p k", p=128))
            nc.sync.dma_start(fc1b_sb[:], fc1b_d[:].rearrange("(k p) -> p k", p=128))
            nc.sync.dma_start(vb_sb[:], vb_d[None, :])
            nc.sync.dma_start(pb_sb[:], pb_d[None, :])
            nc.sync.dma_start(f2b_sb[:], f2b_d[None, :])
            make_identity(nc, ident[:])
            nc.vector.memset(ones_row[:], 1.0)
            nc.vector.memset(ones_f32[:], 1.0)
            nc.vector.memset(eps_sb[:], LN_EPS)

            def ln_transpose(x_tiles, tag, out_tags):
                """LN over feature dim + PE-transpose into pair-packed [128, 2N] bf16 tiles."""
                xT = [b394.tile([128, 2 * N], BF16, tag=out_tags[k], name=f"{tag}T{k}")
                      for k in range(KT)]
                for (e, j), xt in x_tiles.items():
                    toff, tcnt = T_TILES[j]
                    stats = small.tile([128, 3, 6], FP32, tag=f"st_{tag}")
                    mv = small.tile([128, 2], FP32, tag=f"mv_{tag}")
                    sd = small.tile([128, 1], FP32, tag=f"sd_{tag}")
                    rstd = small.tile([128, 1], FP32, tag=f"rs_{tag}")
                    for g in range(3):
                        nc.vector.bn_stats(stats[:tcnt, g, :], xt[:tcnt, g * 256:(g + 1) * 256])
                    nc.vector.bn_aggr(mv[:tcnt], stats[:tcnt])
                    nc.scalar.activation(sd[:tcnt], mv[:tcnt, 1:2], AF.Ln, bias=eps_sb[:tcnt])
                    nc.scalar.activation(rstd[:tcnt], sd[:tcnt], AF.Exp, scale=-0.5)
                    xn = xnp.tile([128, C], BF16, tag="xn")
                    nc.vector.tensor_scalar(
                        xn[:tcnt, :], xt[:tcnt, :],
                        mv[:tcnt, 0:1], rstd[:tcnt, 0:1],
                        op0=AL.subtract, op1=AL.mult)
                    for cb in range(KT):
                        pt = ps_tr.tile([128, 128], BF16, tag="ps_tr")
                        nc.tensor.transpose(
                            pt[:128, :tcnt],
                            xn[:tcnt, cb * 128:(cb + 1) * 128],
                            ident[:tcnt, :tcnt])
                        nc.vector.tensor_copy(
                            xT[cb][:, e * N + toff: e * N + toff + tcnt],
                            pt[:128, :tcnt])
                return xT

            # t-slice within the packed [2N] axis for (e, j)
            def tslice(e, j):
                toff, tcnt = T_TILES[j]
                return e * N + toff, tcnt

            for s in range(npairs):
                # ---------------- load x0 ----------------
                x0 = {}
                for e in range(2):
                    bidx = 2 * s + e
                    for j, (toff, tcnt) in enumerate(T_TILES):
                        t = resid.tile([128, C], FP32, tag=f"x0_{e}{j}", bufs=2 if e == 0 else 1)
                        nc.scalar.dma_start(t[:tcnt, :], x_d[bidx, toff:toff + tcnt, :])
                        x0[(e, j)] = t

                # ---------------- LN1 + transpose ----------------
                xnT = ln_transpose(x0, "ln1", [f"b394_xnT{k}" for k in range(KT)])

                # ---------------- qT, kT ----------------
                qT = [b394.tile([128, 2 * N], BF16, tag=f"b394_qT{ob}", name=f"qT{ob}") for ob in range(KT)]
                kT = [b394.tile([128, 2 * N], BF16, tag=f"b394_kT{ob}", name=f"kT{ob}") for ob in range(KT)]
                for dst, base, bias in ((qT, 0, qb_sb), (kT, C, kb_sb)):
                    for ob in range(KT):
                        ps = ps_mm.tile([128, 2 * N], FP32, tag="ps_mm")
                        for k in range(KT):
                            nc.tensor.matmul(
                                ps[:, :], qkvT[k][:, base + ob * 128: base + (ob + 1) * 128],
                                xnT[k][:, :], start=(k == 0), stop=(k == KT - 1))
                        nc.vector.tensor_scalar_add(dst[ob][:, :], ps[:, :], bias[:, ob:ob + 1])

                # ---------------- v (normal layout, per elem/t-tile) ----------------
                v_sb = {}
                for e in range(2):
                    for j, (toff, tcnt) in enumerate(T_TILES):
                        vt = vpool.tile([128, H, D + 1], BF16, tag="v")
                        nc.vector.memset(vt[:, :, D:D + 1], 1.0)
                        ts_off, ts_cnt = tslice(e, j)
                        for ci, (coff, csz) in enumerate(C_CHUNKS):
                            ps = ps_mm.tile([128, 512], FP32, tag="ps_mm")
                            for k in range(KT):
                                nc.tensor.matmul(
                                    ps[:ts_cnt, :csz],
                                    xnT[k][:, ts_off:ts_off + ts_cnt],
                                    qkvT[k][:, 2 * C + coff: 2 * C + coff + csz],
                                    start=(k == 0), stop=False)
                            nc.tensor.matmul(
                                ps[:ts_cnt, :csz],
                                ones_row[0:1, :ts_cnt],
                                vb_sb[0:1, coff:coff + csz],
                                start=False, stop=True)
                            h0 = coff // D
                            nh = csz // D
                            nc.vector.tensor_copy(
                                vt[:ts_cnt, h0:h0 + nh, 0:D],
                                ps[:ts_cnt, :csz])
                        v_sb[(e, j)] = vt

                # ---------------- attention ----------------
                aT = [b394.tile([128, 2 * N], BF16, tag=f"b394_aT{cb}", name=f"aT{cb}") for cb in range(KT)]
                rpb = (rpb0, rpb1)
                for e in range(2):
                    for h in range(H):
                        hp, hi = divmod(h, 2)
                        rbase = 64 * hi
                        exp_t = []
                        for j2, (tkoff, tkcnt) in enumerate(T_TILES):
                            L = ps_at.tile([128, N], FP32, tag="ps_at")
                            # logitsT[tk, tq] = k_h[tk,:] . q_h[tq,:]
                            nc.tensor.matmul(
                                L[:tkcnt, :N],
                                kT[hp][rbase:rbase + 64, e * N + tkoff: e * N + tkoff + tkcnt],
                                qT[hp][rbase:rbase + 64, e * N: e * N + N],
                                start=True, stop=False)
                            # += rpbT via identity matmul
                            nc.tensor.matmul(
                                L[:tkcnt, :N],
                                ident[:tkcnt, :tkcnt],
                                rpb[j2][:tkcnt, h, :],
                                start=False, stop=True)
                            et = expp.tile([128, N], BF16, tag="exp")
                            nc.scalar.activation(et[:tkcnt, :], L[:tkcnt, :N], AF.Exp)
                            exp_t.append(et)
                        O = ps_at.tile([D + 1, N], FP32, tag="ps_at")
                        for j2, (tkoff, tkcnt) in enumerate(T_TILES):
                            nc.tensor.matmul(
                                O[:D + 1, :N],
                                v_sb[(e, j2)][:tkcnt, h, :],
                                exp_t[j2][:tkcnt, :N],
                                start=(j2 == 0), stop=(j2 == 1))
                        lden = small.tile([1, N], FP32, tag="lden", name="lden")
                        r = small.tile([1, N], FP32, tag="recip", name="r")
                        nc.scalar.activation(lden[:, :], O[D:D + 1, :N], AF.Ln)
                        nc.scalar.activation(r[:, :], lden[:, :], AF.Exp, scale=-1.0)
                        Dn = ps_at.tile([64, N], FP32, tag="ps_at")
                        nc.tensor.matmul(Dn[:, :], ones_f32[0:1, 0:64], r[0:1, :])
                        Dsb = dpool.tile([64, N], FP32, tag="D")
                        nc.scalar.copy(Dsb[:, :], Dn[:, :])
                        nc.vector.tensor_tensor(
                            aT[hp][rbase:rbase + 64, e * N: e * N + N],
                            O[0:D, :N], Dsb[:, :], op=AL.mult)

                # ---------------- proj + residual -> x1 ----------------
                x1 = {}
                for e in range(2):
                    for j, (toff, tcnt) in enumerate(T_TILES):
                        xt = resid.tile([128, C], FP32, tag=f"x1_{e}{j}")
                        ts_off, ts_cnt = tslice(e, j)
                        for ci, (coff, csz) in enumerate(C_CHUNKS):
                            ps = ps_mm.tile([128, 512], FP32, tag="ps_mm")
                            for k in range(KT):
                                nc.tensor.matmul(
                                    ps[:ts_cnt, :csz],
                                    aT[k][:, ts_off:ts_off + ts_cnt],
                                    projT[k][:, coff:coff + csz],
                                    start=(k == 0), stop=False)
                            nc.tensor.matmul(
                                ps[:ts_cnt, :csz],
                                ones_row[0:1, :ts_cnt],
                                pb_sb[0:1, coff:coff + csz],
                                start=False, stop=True)
                            nc.vector.tensor_tensor(
                                xt[:ts_cnt, coff:coff + csz],
                                ps[:ts_cnt, :csz],
                                x0[(e, j)][:ts_cnt, coff:coff + csz], op=AL.add)
                        x1[(e, j)] = xt

                # ---------------- LN2 + transpose ----------------
                hnT = ln_transpose(x1, "ln2", [f"b394_hnT{k}" for k in range(KT)])

                # ---------------- fc1 + gelu -> hT ----------------
                _ht_tags = ([f"b394_xnT{k}" for k in range(KT)] + [f"b394_qT{k}" for k in range(KT)] + [f"b394_kT{k}" for k in range(KT)] + [f"b394_aT{k}" for k in range(KT)])
                hT = [b394.tile([128, 2 * N], BF16, tag=_ht_tags[ob], name=f"hT{ob}") for ob in range(KT2)]
                for ob in range(KT2):
                    ps = ps_mm.tile([128, 2 * N], FP32, tag="ps_mm")
                    for k in range(KT):
                        nc.tensor.matmul(
                            ps[:, :], fc1T[k][:, ob * 128:(ob + 1) * 128],
                            hnT[k][:, :], start=(k == 0), stop=(k == KT - 1))
                    nc.scalar.activation(
                        hT[ob][:, :], ps[:, :], AF.Gelu,
                        bias=fc1b_sb[:, ob:ob + 1])

                # ---------------- fc2 + residual -> y ----------------
                for e in range(2):
                    bidx = 2 * s + e
                    for j, (toff, tcnt) in enumerate(T_TILES):
                        ot = resid.tile([128, C], FP32, tag=f"x0_{e}{j}", name=f"out_{e}{j}", bufs=2 if e == 0 else 1)
                        ts_off, ts_cnt = tslice(e, j)
                        for ci, (coff, csz) in enumerate(C_CHUNKS):
                            ps = ps_mm.tile([128, 512], FP32, tag="ps_mm")
                            for k in range(KT2):
                                nc.tensor.matmul(
                                    ps[:ts_cnt, :csz],
                                    hT[k][:, ts_off:ts_off + ts_cnt],
                                    fc2T[k][:, coff:coff + csz],
                                    start=(k == 0), stop=False)
                            nc.tensor.matmul(
                                ps[:ts_cnt, :csz],
                                ones_row[0:1, :ts_cnt],
                                f2b_sb[0:1, coff:coff + csz],
                                start=False, stop=True)
                            nc.vector.tensor_tensor(
                                ot[:ts_cnt, coff:coff + csz],
                                ps[:ts_cnt, :csz],
                                x1[(e, j)][:ts_cnt, coff:coff + csz], op=AL.add)
                        nc.gpsimd.dma_start(y_d[bidx, toff:toff + tcnt, :], ot[:tcnt, :])

    return nc


def fold_weights(inputs):
    """Host-side folding. Returns dict of per-core-shared input arrays."""
    import ml_dtypes
    f32 = np.float32
    bf16 = ml_dtypes.bfloat16
    g = {k: np.asarray(v) for k, v in inputs.items()}
    n1w, n1b = g["n1_w"].astype(f32), g["n1_b"].astype(f32)
    n2w, n2b = g["n2_w"].astype(f32), g["n2_b"].astype(f32)
    g1, g2 = g["gamma1"].astype(f32), g["gamma2"].astype(f32)
    qkv_w = g["qkv_w"].astype(f32)
    q_bias, v_bias = g["q_bias"].astype(f32), g["v_bias"].astype(f32)
    proj_w, proj_b = g["proj_w"].astype(f32), g["proj_b"].astype(f32)
    fc1_w, fc1_b = g["fc1_w"].astype(f32), g["fc1_b"].astype(f32)
    fc2_w, fc2_b = g["fc2_w"].astype(f32), g["fc2_b"].astype(f32)

    qkv_bias = np.concatenate([q_bias, np.zeros_like(q_bias), v_bias])
    Wq = qkv_w * n1w[None, :]
    bq = qkv_bias + qkv_w @ n1b
    scale = (C // H) ** -0.5
    Wq[:C] *= scale
    bq[:C] *= scale

    Pw = g1[:, None] * proj_w
    pb = g1 * proj_b
    F1 = fc1_w * n2w[None, :]
    f1b = fc1_b + fc1_w @ n2b
    F2 = g2[:, None] * fc2_w
    f2b = g2 * fc2_b

    table = g["rel_bias_table"].astype(f32)
    idx = np.asarray(g["rel_index"]).reshape(-1)
    rpb_ref = table[idx].reshape(N, N, H).transpose(2, 0, 1)  # [h, tq, tk]
    rpbT = rpb_ref.transpose(0, 2, 1)                          # [h, tk, tq]
    rpb0 = np.ascontiguousarray(rpbT[:, :128, :].transpose(1, 0, 2)).astype(bf16)
    rpb1 = np.ascontiguousarray(rpbT[:, 128:, :].transpose(1, 0, 2)).astype(bf16)

    return {
        "qkvT": np.ascontiguousarray(Wq.T).astype(bf16),
        "projT": np.ascontiguousarray(Pw.T).astype(bf16),
        "fc1T": np.ascontiguousarray(F1.T).astype(bf16),
        "fc2T": np.ascontiguousarray(F2.T).astype(bf16),
        "rpb0": rpb0,
        "rpb1": rpb1,
        "qb": np.ascontiguousarray(bq[:C]),
        "kb": np.ascontiguousarray(bq[C:2 * C]),
        "fc1b": f1b,
        "vbrow": bq[2 * C:].astype(bf16),
        "pbrow": pb.astype(bf16),
        "f2brow": f2b.astype(bf16),
    }


_CACHE = {}


def _get_nc():
    if "nc" not in _CACHE:
        nc = build_nc()
        patched = _legalize_waits(nc.to_json_bytes())
        nc.to_json_bytes = lambda: patched
        _CACHE["nc"] = nc
    return _CACHE["nc"]


def kernel(**inputs):
    from concourse.bass_utils import run_bass_kernel_spmd
    nc = _get_nc()
    folded = fold_weights(inputs)
    x = np.ascontiguousarray(np.asarray(inputs["x"], dtype=np.float32))
    assert x.shape == (B, N, C), x.shape
    in_maps = []
    for c in range(NCORES):
        m = dict(folded)
        m["x"] = np.ascontiguousarray(x[c * BPC:(c + 1) * BPC])
        in_maps.append(m)
    res = run_bass_kernel_spmd(nc, in_maps, core_ids=list(range(NCORES)))
    out = np.concatenate([res.results[c]["y"] for c in range(NCORES)], axis=0)
    return out.astype(np.float32)



# revision 9
# speedup vs baseline: 1.2973x; 1.2973x over previous
"""Trainium2 Bass kernel for nn_Block_74363063763569 (BEiT-style transformer block).

Data-parallel over batch across 8 NeuronCores (8 elems/core), zero collectives.
fp8e4m3 DoubleRow GEMMs; see build_nc docstring for the numerics scheme.
"""
import sys, json
sys.path.insert(0, "/opt/trn_rl_repo")
import numpy as np


def _legalize_waits(bir_bytes, max_waits=1):
    """This container's walrus rejects >1 sync wait per instruction; split
    extras into preceding single-wait EventSemaphore instructions."""
    j = json.loads(bir_bytes)
    for f in j["functions"]:
        for b in f["blocks"]:
            out = []
            for inst in b["instructions"]:
                si = inst.get("sync_info")
                waits = si.get("on_wait", []) if si else []
                if len(waits) > max_waits:
                    keep, extra = waits[:max_waits], waits[max_waits:]
                    for k, w in enumerate(extra):
                        out.append({"debug": inst.get("debug", 0), "engine": inst["engine"],
                                    "ins": [], "name": f"{inst['name']}_w{k}",
                                    "opcode": "EventSemaphore", "outs": [],
                                    "sync_info": {"on_update": [], "on_wait": [w]}})
                    si["on_wait"] = keep
                out.append(inst)
            b["instructions"] = out
    return json.dumps(j).encode()


import concourse.bass as bass
import concourse.tile as tile
import concourse.mybir as mybir
from concourse.masks import make_identity

FP32 = mybir.dt.float32
BF16 = mybir.dt.bfloat16
FP8 = mybir.dt.float8e4
DR = mybir.MatmulPerfMode.DoubleRow

B = 64
N = 197
C = 768
H = 12
D = 64
HID = 3072
NCORES = 8
BPC = B // NCORES
NPAIRS_FULL = BPC // 2
KG = C // 256      # 3 doublerow contraction groups over C
KG2 = HID // 256   # 12 groups over HID
LN_EPS = 1e-5

R = 256.0    # residual stream scale
SW = 64.0    # qkv/fc1 weight scale
SQ = 8.0     # q/k fp8 activation scale
SV = 8.0     # v fp8 activation scale (ones col = SV)

T_TILES = [(0, 128), (128, 69)]
T_PADS = [128, 72]           # padded token counts for fp8 stationary slices
EN = 200                     # padded per-elem token stride (4-aligned offsets)
TT = 2 * EN                  # packed token extent
C_CHUNKS = [(0, 512), (512, 256)]

AL = mybir.AluOpType
AF = mybir.ActivationFunctionType


def build_nc(npairs=NPAIRS_FULL):
    nb = 2 * npairs
    nc = bass.Bass()

    x_d = nc.dram_tensor("x", [nb, N, C], FP32, kind="ExternalInput")
    wqk_d = nc.dram_tensor("wqk", [KG, 128, 2, 1536], FP8, kind="ExternalInput")
    wv_d = nc.dram_tensor("wv", [KG, 128, 2, C], FP8, kind="ExternalInput")
    wvb_d = nc.dram_tensor("wvb", [1, 2, C], FP8, kind="ExternalInput")
    wp_d = nc.dram_tensor("wp", [KG, 128, 2, C], FP8, kind="ExternalInput")
    wf1_d = nc.dram_tensor("wf1", [KG, 128, 2, HID], FP8, kind="ExternalInput")
    wf2_d = nc.dram_tensor("wf2", [KG2, 128, 2, C], FP8, kind="ExternalInput")
    qkb_d = nc.dram_tensor("qkb", [128, 12], FP32, kind="ExternalInput")
    f1b_d = nc.dram_tensor("f1b", [128, 24], FP32, kind="ExternalInput")
    erpb0_d = nc.dram_tensor("erpb0", [128, H, N], BF16, kind="ExternalInput")
    erpb1_d = nc.dram_tensor("erpb1", [69, H, N], BF16, kind="ExternalInput")
    y_d = nc.dram_tensor("y", [nb, N, C], FP32, kind="ExternalOutput")

    with tile.TileContext(nc) as tc:
        with (
            tc.tile_pool(name="singles", bufs=1) as singles,
            tc.tile_pool(name="resid", bufs=2) as resid,
            tc.tile_pool(name="act", bufs=1) as act,      # per-pair fp8 activations
            tc.tile_pool(name="xn", bufs=3) as xnp,
            tc.tile_pool(name="expp", bufs=3) as expp,
            tc.tile_pool(name="small", bufs=8) as small,
            tc.tile_pool(name="ps_tr", bufs=2, space="PSUM") as ps_tr,
            tc.tile_pool(name="ps_mm", bufs=2, space="PSUM") as ps_mm,
            tc.tile_pool(name="ps_at", bufs=4, space="PSUM") as ps_at,
        ):
            # ---- persistent weights / constants ----
            wqk = [singles.tile([128, 2, 1536], FP8, tag=f"wqk{g}") for g in range(KG)]
            wv = [singles.tile([128, 2, C], FP8, tag=f"wv{g}") for g in range(KG)]
            wvb = singles.tile([1, 2, C], FP8, tag="wvb")
            wp = [singles.tile([128, 2, C], FP8, tag=f"wp{g}") for g in range(KG)]
            wf1 = [singles.tile([128, 2, HID], FP8, tag=f"wf1{g}") for g in range(KG)]
            wf2 = [singles.tile([128, 2, C], FP8, tag=f"wf2{g}") for g in range(KG2)]
            qkb = singles.tile([128, 12], FP32, tag="qkb")
            f1b = singles.tile([128, 24], FP32, tag="f1b")
            erpb = [singles.tile([128, H, N], BF16, tag="erpb0"),
                    singles.tile([69, H, N], BF16, tag="erpb1")]
            ident = singles.tile([128, 128], BF16, tag="ident")
            ones_x = singles.tile([1, 2, 128], FP8, tag="ones_x")
            ones_bf = singles.tile([1, 64], BF16, tag="ones_bf")
            eps_sb = singles.tile([128, 1], FP32, tag="eps")

            for g in range(KG):
                nc.sync.dma_start(wqk[g][:], wqk_d[g])
                nc.sync.dma_start(wv[g][:], wv_d[g])
                nc.sync.dma_start(wp[g][:], wp_d[g])
                nc.sync.dma_start(wf1[g][:], wf1_d[g])
            for g in range(KG2):
                nc.sync.dma_start(wf2[g][:], wf2_d[g])
            nc.sync.dma_start(wvb[:], wvb_d[:])
            nc.sync.dma_start(qkb[:], qkb_d[:])
            nc.sync.dma_start(f1b[:], f1b_d[:])
            nc.sync.dma_start(erpb[0][:], erpb0_d[:])
            nc.sync.dma_start(erpb[1][:], erpb1_d[:])
            make_identity(nc, ident[:])
            nc.vector.memset(ones_x[:], 1.0)
            nc.vector.memset(ones_bf[:], 1.0)
            nc.vector.memset(eps_sb[:], LN_EPS)

            def ln_transpose(x_tiles, tag):
                """LN over features + fp8 transpose into [128, 2, 2N] group tiles."""
                xT = [act.tile([128, 2, 2 * N], FP8, tag=f"{tag}T{g}") for g in range(KG)]
                for (e, j), xt in x_tiles.items():
                    toff, tcnt = T_TILES[j]
                    stats = small.tile([128, 3, 6], FP32, tag=f"st_{tag}")
                    mv = small.tile([128, 2], FP32, tag=f"mv_{tag}")
                    sd = small.tile([128, 1], FP32, tag=f"sd_{tag}")
                    rstd = small.tile([128, 1], FP32, tag=f"rs_{tag}")
                    for g3 in range(3):
                        nc.vector.bn_stats(stats[:tcnt, g3, :], xt[:tcnt, g3 * 256:(g3 + 1) * 256])
                    nc.vector.bn_aggr(mv[:tcnt], stats[:tcnt])
                    nc.scalar.activation(sd[:tcnt], mv[:tcnt, 1:2], AF.Ln, bias=eps_sb[:tcnt])
                    nc.scalar.activation(rstd[:tcnt], sd[:tcnt], AF.Exp, scale=-0.5)
                    xn8 = xnp.tile([128, C], BF16, tag="xn8")
                    nc.vector.tensor_scalar(
                        xn8[:tcnt, :], xt[:tcnt, :],
                        mv[:tcnt, 0:1], rstd[:tcnt, 0:1],
                        op0=AL.subtract, op1=AL.mult)
                    for cb in range(6):
                        pt = ps_tr.tile([128, 128], BF16, tag="ps_tr")
                        nc.tensor.transpose(
                            pt[:128, :tcnt],
                            xn8[:tcnt, cb * 128:(cb + 1) * 128],
                            ident[:tcnt, :tcnt])
                        dst = xT[cb // 2][:, cb % 2, e * EN + toff: e * EN + toff + tcnt]
                        if cb % 2 == 0:
                            nc.scalar.copy(dst, pt[:128, :tcnt])
                        else:
                            nc.vector.tensor_copy(dst, pt[:128, :tcnt])
                return xT

            for s in range(npairs):
                # ---------------- load x0 ----------------
                x0 = {}
                for e in range(2):
                    bidx = 2 * s + e
                    for j, (toff, tcnt) in enumerate(T_TILES):
                        t = resid.tile([128, C], FP32, tag=f"x0_{e}{j}")
                        nc.scalar.dma_start(t[:tcnt, :], x_d[bidx, toff:toff + tcnt, :])
                        x0[(e, j)] = t

                # ---------------- LN1 + fp8 transpose ----------------
                xnT = ln_transpose(x0, "ln1")

                # ---------------- q,k (weight-stationary DR) ----------------
                qkT = [act.tile([128, 2 * N], FP8, tag=f"qkT{b}") for b in range(12)]
                for b in range(12):
                    ps = ps_mm.tile([128, TT], FP32, tag="ps_mm")
                    for g in range(KG):
                        nc.tensor.matmul(
                            ps[:, :], wqk[g][:, :, b * 128:(b + 1) * 128],
                            xnT[g][:, :, :], start=(g == 0), stop=(g == KG - 1),
                            perf_mode=DR)
                    nc.vector.tensor_scalar(
                        qkT[b][:, :], ps[:, :], SQ / SW, qkb[:, b:b + 1],
                        op0=AL.mult, op1=AL.add)

                # ---------------- v (act-stationary DR) + vb row ----------------
                vt = {}
                for e in range(2):
                    v8 = act.tile([128, 2, H, 68], FP8, tag=f"vt{e}")
                    nc.vector.memset(v8[:, :, :, :], 0.0)
                    nc.vector.memset(v8[:, :, :, D:D + 1], SV)
                    vt[e] = v8
                for e in range(2):
                    for j, (toff, tcnt) in enumerate(T_TILES):
                        tp = T_PADS[j]
                        ts = e * EN + toff
                        for ci, (coff, csz) in enumerate(C_CHUNKS):
                            nh = csz // D
                            h0 = coff // D
                            ps = ps_mm.tile([128, 8, D], FP32, tag="ps_mm")
                            nc.tensor.matmul(
                                ps[:tp, :nh, :],
                                ones_x[:, :, :tp],
                                wvb[:, :, coff:coff + csz],
                                start=True, stop=False, perf_mode=DR)
                            for g in range(KG):
                                nc.tensor.matmul(
                                    ps[:tp, :nh, :],
                                    xnT[g][:, :, ts:ts + tp],
                                    wv[g][:, :, coff:coff + csz],
                                    start=False, stop=(g == KG - 1), perf_mode=DR)
                            nc.vector.tensor_scalar(
                                vt[e][:tcnt, j, h0:h0 + nh, 0:D],
                                ps[:tcnt, :nh, :], SV / SW, None, op0=AL.mult)

                # ---------------- attention ----------------
                aT = [act.tile([128, 2, 2 * N], FP8, tag=f"aT{g}") for g in range(KG)]
                for e in range(2):
                    for h in range(H):
                        qrow = 64 * (h % 2)
                        qt = qkT[h // 2]
                        kt = qkT[6 + h // 2]
                        et = expp.tile([128, 2, EN], FP8, tag="et")
                        nc.vector.memset(et[64:, 1, :], 0.0)
                        nc.vector.memset(et[:, :, N:], 0.0)
                        for j2, (tkoff, tkcnt) in enumerate(T_TILES):
                            tkp = T_PADS[j2]
                            L = ps_at.tile([128, EN], FP32, tag="ps_at")
                            nc.tensor.matmul(
                                L[:tkp, :EN],
                                kt[qrow:qrow + 64, e * EN + tkoff: e * EN + tkoff + tkp],
                                qt[qrow:qrow + 64, e * EN: e * EN + EN])
                            ex = expp.tile([128, N], BF16, tag="ex")
                            nc.scalar.activation(ex[:tkcnt, :], L[:tkcnt, :N],
                                                 AF.Exp, scale=1.0 / (SQ * SQ))
                            nc.vector.tensor_tensor(
                                et[:tkcnt, j2, :N], ex[:tkcnt, :],
                                erpb[j2][:tkcnt, h, :], op=AL.mult)
                        O = ps_at.tile([68, EN], FP32, tag="ps_at")
                        nc.tensor.matmul(
                            O[:68, :EN],
                            vt[e][:, :, h, :],
                            et[:, :, :], perf_mode=DR)
                        rb = small.tile([1, N], BF16, tag="recip")
                        with nc.allow_low_precision(reason="softmax recip bf16"):
                            nc.vector.reciprocal(rb[:, :], O[D:D + 1, :N])
                        Dn = ps_at.tile([64, N], FP32, tag="ps_at")
                        nc.tensor.matmul(Dn[:, :], ones_bf[0:1, :], rb[0:1, :])
                        Dsb = expp.tile([64, N], BF16, tag="Dsb")
                        nc.scalar.copy(Dsb[:, :], Dn[:, :])
                        nc.vector.tensor_tensor(
                            aT[h // 4][64 * (h % 2):64 * (h % 2) + 64, (h % 4) // 2,
                                       e * EN: e * EN + N],
                            O[0:D, :N], Dsb[:, :], op=AL.mult)

                # ---------------- proj + residual -> x1 ----------------
                x1 = {}
                for e in range(2):
                    for j, (toff, tcnt) in enumerate(T_TILES):
                        tp = T_PADS[j]
                        xt = resid.tile([128, C], FP32, tag=f"x1_{e}{j}")
                        ts = e * EN + toff
                        for ci, (coff, csz) in enumerate(C_CHUNKS):
                            ps = ps_mm.tile([128, 512], FP32, tag="ps_mm")
                            for g in range(KG):
                                nc.tensor.matmul(
                                    ps[:tp, :csz],
                                    aT[g][:, :, ts:ts + tp],
                                    wp[g][:, :, coff:coff + csz],
                                    start=(g == 0), stop=(g == KG - 1), perf_mode=DR)
                            nc.vector.tensor_tensor(
                                xt[:tcnt, coff:coff + csz],
                                ps[:tcnt, :csz],
                                x0[(e, j)][:tcnt, coff:coff + csz], op=AL.add)
                        x1[(e, j)] = xt

                # ---------------- LN2 + fp8 transpose ----------------
                hnT = ln_transpose(x1, "ln2")

                # ---------------- fc1 + gelu -> hT ----------------
                hT = [act.tile([128, 2, 2 * N], FP8, tag=f"hT{g}") for g in range(KG2)]
                for ob in range(24):
                    ps = ps_mm.tile([128, TT], FP32, tag="ps_mm")
                    for g in range(KG):
                        nc.tensor.matmul(
                            ps[:, :], wf1[g][:, :, ob * 128:(ob + 1) * 128],
                            hnT[g][:, :, :], start=(g == 0), stop=(g == KG - 1),
                            perf_mode=DR)
                    nc.scalar.activation(
                        hT[ob // 2][:, ob % 2, :], ps[:, :], AF.Gelu,
                        scale=1.0 / SW, bias=f1b[:, ob:ob + 1])

                # ---------------- fc2 + residual -> y ----------------
                for e in range(2):
                    bidx = 2 * s + e
                    for j, (toff, tcnt) in enumerate(T_TILES):
                        tp = T_PADS[j]
                        ot = resid.tile([128, C], FP32, tag=f"out_{e}{j}")
                        ts = e * EN + toff
                        for ci, (coff, csz) in enumerate(C_CHUNKS):
                            ps = ps_mm.tile([128, 512], FP32, tag="ps_mm")
                            for g in range(KG2):
                                nc.tensor.matmul(
                                    ps[:tp, :csz],
                                    hT[g][:, :, ts:ts + tp],
                                    wf2[g][:, :, coff:coff + csz],
                                    start=(g == 0), stop=(g == KG2 - 1), perf_mode=DR)
                            nc.vector.tensor_tensor(
                                ot[:tcnt, coff:coff + csz],
                                ps[:tcnt, :csz],
                                x1[(e, j)][:tcnt, coff:coff + csz], op=AL.add)
                        nc.gpsimd.dma_start(y_d[bidx, toff:toff + tcnt, :], ot[:tcnt, :])

    return nc


def fold_weights(inputs):
    """Host-side folding into fp8 DoubleRow layouts. Returns per-core dict."""
    import ml_dtypes
    f8 = ml_dtypes.float8_e4m3
    bf = ml_dtypes.bfloat16
    f32 = np.float32
    g = {k: np.asarray(v) for k, v in inputs.items()}
    n1w, n1b = g["n1_w"].astype(f32), g["n1_b"].astype(f32)
    n2w, n2b = g["n2_w"].astype(f32), g["n2_b"].astype(f32)
    g1, g2 = g["gamma1"].astype(f32), g["gamma2"].astype(f32)
    qkv_w = g["qkv_w"].astype(f32)
    q_bias, v_bias = g["q_bias"].astype(f32), g["v_bias"].astype(f32)
    proj_w, proj_b = g["proj_w"].astype(f32), g["proj_b"].astype(f32)
    fc1_w, fc1_b = g["fc1_w"].astype(f32), g["fc1_b"].astype(f32)
    fc2_w, fc2_b = g["fc2_w"].astype(f32), g["fc2_b"].astype(f32)
    assert np.all(proj_b == 0), "kernel assumes proj_b == 0"

    qkv_bias = np.concatenate([q_bias, np.zeros_like(q_bias), v_bias])
    Wq = qkv_w * n1w[None, :]                       # LN affine fold
    bq = qkv_bias + qkv_w @ n1b
    scale = D ** -0.5
    Wq[:C] *= scale
    bq[:C] *= scale

    def dr_pack(wT, ngroups):
        # wT: [in_features, out] -> [ngroups, 128, 2, out]
        nin = wT.shape[0]
        assert nin == ngroups * 256
        return np.ascontiguousarray(
            wT.reshape(ngroups, 2, 128, -1).transpose(0, 2, 1, 3))

    wqk = dr_pack((SW * Wq[:2 * C].T), KG).astype(f8)
    wv = dr_pack((SW * Wq[2 * C:].T), KG).astype(f8)
    wvb = np.zeros((1, 2, C), np.float32)
    wvb[0, 0, :] = SW * bq[2 * C:]
    Pw = (g1[:, None] * proj_w)
    wp = dr_pack((R * Pw.T), KG).astype(f8)
    F1 = fc1_w * n2w[None, :]
    f1b_full = fc1_b + fc1_w @ n2b
    wf1 = dr_pack((SW * F1.T), KG).astype(f8)
    F2 = g2[:, None] * fc2_w
    wf2 = dr_pack((R * F2.T), KG2).astype(f8)

    qkb = (SQ * bq[:2 * C]).reshape(12, 128).T.copy()
    f1b = f1b_full.reshape(24, 128).T.copy()

    table = g["rel_bias_table"].astype(f32)
    idx = np.asarray(g["rel_index"]).reshape(-1)
    rpb = table[idx].reshape(N, N, H).transpose(2, 0, 1)   # [h, tq, tk]
    rpbT = rpb.transpose(0, 2, 1)                          # [h, tk, tq]
    erpb = np.exp(rpbT)
    erpb0 = np.ascontiguousarray(erpb[:, :128, :].transpose(1, 0, 2)).astype(bf)
    erpb1 = np.ascontiguousarray(erpb[:, 128:, :].transpose(1, 0, 2)).astype(bf)

    return {
        "wqk": wqk, "wv": wv, "wvb": wvb.astype(f8), "wp": wp,
        "wf1": wf1, "wf2": wf2,
        "qkb": np.ascontiguousarray(qkb), "f1b": np.ascontiguousarray(f1b),
        "erpb0": erpb0, "erpb1": erpb1,
    }, (g2 * fc2_b).astype(f32)


_CACHE = {}


def _get_nc():
    if "nc" not in _CACHE:
        nc = build_nc()
        patched = _legalize_waits(nc.to_json_bytes())
        nc.to_json_bytes = lambda: patched
        _CACHE["nc"] = nc
    return _CACHE["nc"]


def kernel(**inputs):
    from concourse.bass_utils import run_bass_kernel_spmd
    nc = _get_nc()
    folded, f2b_host = fold_weights(inputs)
    x = np.ascontiguousarray(np.asarray(inputs["x"], dtype=np.float32))
    assert x.shape == (B, N, C), x.shape
    xs = R * x
    in_maps = []
    for c in range(NCORES):
        m = dict(folded)
        m["x"] = np.ascontiguousarray(xs[c * BPC:(c + 1) * BPC])
        in_maps.append(m)
    res = run_bass_kernel_spmd(nc, in_maps, core_ids=list(range(NCORES)))
    out = np.concatenate([res.results[c]["y"] for c in range(NCORES)], axis=0)
    return (out * (1.0 / R) + f2b_host).astype(np.float32)


# revision 12
# speedup vs baseline: 1.3589x; 1.0475x over previous
"""Trainium2 Bass kernel for nn_Block_74363063763569 (BEiT-style transformer block).

Data-parallel over batch across 8 NeuronCores (8 elems/core), zero collectives.
fp8e4m3 DoubleRow GEMMs; see build_nc docstring for the numerics scheme.
"""
import sys, json
sys.path.insert(0, "/opt/trn_rl_repo")
import numpy as np


def _legalize_waits(bir_bytes, max_waits=1):
    """This container's walrus rejects >1 sync wait per instruction; split
    extras into preceding single-wait EventSemaphore instructions."""
    j = json.loads(bir_bytes)
    for f in j["functions"]:
        for b in f["blocks"]:
            out = []
            for inst in b["instructions"]:
                si = inst.get("sync_info")
                waits = si.get("on_wait", []) if si else []
                if len(waits) > max_waits:
                    keep, extra = waits[:max_waits], waits[max_waits:]
                    for k, w in enumerate(extra):
                        out.append({"debug": inst.get("debug", 0), "engine": inst["engine"],
                                    "ins": [], "name": f"{inst['name']}_w{k}",
                                    "opcode": "EventSemaphore", "outs": [],
                                    "sync_info": {"on_update": [], "on_wait": [w]}})
                    si["on_wait"] = keep
                out.append(inst)
            b["instructions"] = out
    return json.dumps(j).encode()


import concourse.bass as bass
import concourse.tile as tile
import concourse.mybir as mybir
from concourse.masks import make_identity

FP32 = mybir.dt.float32
BF16 = mybir.dt.bfloat16
FP8 = mybir.dt.float8e4
DR = mybir.MatmulPerfMode.DoubleRow

B = 64
N = 197
C = 768
H = 12
D = 64
HID = 3072
NCORES = 8
BPC = B // NCORES
NPAIRS_FULL = BPC // 2
KG = C // 256      # 3 doublerow contraction groups over C
KG2 = HID // 256   # 12 groups over HID
LN_EPS = 1e-5

R = 256.0    # residual stream scale
SW = 64.0    # qkv/fc1 weight scale
SQ = 8.0     # q/k fp8 activation scale
SV = 8.0     # v fp8 activation scale (ones col = SV)

T_TILES = [(0, 128), (128, 69)]
T_PADS = [128, 72]           # padded token counts for fp8 stationary slices
EN = 200                     # padded per-elem token stride (4-aligned offsets)
TT = 2 * EN                  # packed token extent
C_CHUNKS = [(0, 512), (512, 256)]

AL = mybir.AluOpType
AF = mybir.ActivationFunctionType


def build_nc(npairs=NPAIRS_FULL):
    nb = 2 * npairs
    nc = bass.Bass()

    x_d = nc.dram_tensor("x", [nb, N, C], FP32, kind="ExternalInput")
    wqk_d = nc.dram_tensor("wqk", [KG, 128, 2, 1536], FP8, kind="ExternalInput")
    wv_d = nc.dram_tensor("wv", [KG, 128, 2, C], FP8, kind="ExternalInput")
    wvb_d = nc.dram_tensor("wvb", [1, 2, C], FP8, kind="ExternalInput")
    wp_d = nc.dram_tensor("wp", [KG, 128, 2, C], FP8, kind="ExternalInput")
    wf1_d = nc.dram_tensor("wf1", [KG, 128, 2, HID], FP8, kind="ExternalInput")
    wf2_d = nc.dram_tensor("wf2", [KG2, 128, 2, C], FP8, kind="ExternalInput")
    qkb_d = nc.dram_tensor("qkb", [128, 12], FP32, kind="ExternalInput")
    f1b_d = nc.dram_tensor("f1b", [128, 24], FP32, kind="ExternalInput")
    erpb0_d = nc.dram_tensor("erpb0", [128, H, N], BF16, kind="ExternalInput")
    erpb1_d = nc.dram_tensor("erpb1", [69, H, N], BF16, kind="ExternalInput")
    y_d = nc.dram_tensor("y", [nb, N, C], FP32, kind="ExternalOutput")

    with tile.TileContext(nc) as tc:
        with (
            tc.tile_pool(name="singles", bufs=1) as singles,
            tc.tile_pool(name="resid", bufs=2) as resid,
            tc.tile_pool(name="act", bufs=1) as act,      # per-pair fp8 activations
            tc.tile_pool(name="xn", bufs=3) as xnp,
            tc.tile_pool(name="expp", bufs=3) as expp,
            tc.tile_pool(name="small", bufs=8) as small,
            tc.tile_pool(name="ps_tr", bufs=2, space="PSUM") as ps_tr,
            tc.tile_pool(name="ps_mm", bufs=2, space="PSUM") as ps_mm,
            tc.tile_pool(name="ps_at", bufs=4, space="PSUM") as ps_at,
        ):
            # ---- persistent weights / constants ----
            wqk = [singles.tile([128, 2, 1536], FP8, tag=f"wqk{g}") for g in range(KG)]
            wv = [singles.tile([128, 2, C], FP8, tag=f"wv{g}") for g in range(KG)]
            wvb = singles.tile([1, 2, C], FP8, tag="wvb")
            wp = [singles.tile([128, 2, C], FP8, tag=f"wp{g}") for g in range(KG)]
            wf1 = [singles.tile([128, 2, HID], FP8, tag=f"wf1{g}") for g in range(KG)]
            wf2 = [singles.tile([128, 2, C], FP8, tag=f"wf2{g}") for g in range(KG2)]
            qkb = singles.tile([128, 12], FP32, tag="qkb")
            f1b = singles.tile([128, 24], FP32, tag="f1b")
            erpb = [singles.tile([128, H, N], BF16, tag="erpb0"),
                    singles.tile([69, H, N], BF16, tag="erpb1")]
            ident = singles.tile([128, 128], BF16, tag="ident")
            ones_x = singles.tile([1, 2, 128], FP8, tag="ones_x")
            ones_bf = singles.tile([1, 64], BF16, tag="ones_bf")
            eps_sb = singles.tile([128, 1], FP32, tag="eps")

            for g in range(KG):
                nc.sync.dma_start(wqk[g][:], wqk_d[g])
                nc.sync.dma_start(wv[g][:], wv_d[g])
                nc.sync.dma_start(wp[g][:], wp_d[g])
                nc.sync.dma_start(wf1[g][:], wf1_d[g])
            for g in range(KG2):
                nc.sync.dma_start(wf2[g][:], wf2_d[g])
            nc.sync.dma_start(wvb[:], wvb_d[:])
            nc.sync.dma_start(qkb[:], qkb_d[:])
            nc.sync.dma_start(f1b[:], f1b_d[:])
            nc.sync.dma_start(erpb[0][:], erpb0_d[:])
            nc.sync.dma_start(erpb[1][:], erpb1_d[:])
            make_identity(nc, ident[:])
            nc.vector.memset(ones_x[:], 1.0)
            nc.vector.memset(ones_bf[:], 1.0)
            nc.vector.memset(eps_sb[:], LN_EPS)

            def ln_transpose(x_tiles, tag):
                """LN over features + fp8 transpose into [128, 2, 2N] group tiles."""
                xT = [act.tile([128, 2, 2 * N], FP8, tag=f"{tag}T{g}") for g in range(KG)]
                for (e, j), xt in x_tiles.items():
                    toff, tcnt = T_TILES[j]
                    stats = small.tile([128, 3, 6], FP32, tag=f"st_{tag}")
                    mv = small.tile([128, 2], FP32, tag=f"mv_{tag}")
                    sd = small.tile([128, 1], FP32, tag=f"sd_{tag}")
                    rstd = small.tile([128, 1], FP32, tag=f"rs_{tag}")
                    for g3 in range(3):
                        nc.vector.bn_stats(stats[:tcnt, g3, :], xt[:tcnt, g3 * 256:(g3 + 1) * 256])
                    nc.vector.bn_aggr(mv[:tcnt], stats[:tcnt])
                    nc.scalar.activation(sd[:tcnt], mv[:tcnt, 1:2], AF.Ln, bias=eps_sb[:tcnt])
                    nc.scalar.activation(rstd[:tcnt], sd[:tcnt], AF.Exp, scale=-0.5)
                    xn8 = xnp.tile([128, C], BF16, tag="xn8")
                    nc.vector.tensor_scalar(
                        xn8[:tcnt, :], xt[:tcnt, :],
                        mv[:tcnt, 0:1], rstd[:tcnt, 0:1],
                        op0=AL.subtract, op1=AL.mult)
                    for cb in range(6):
                        pt = ps_tr.tile([128, 128], BF16, tag="ps_tr")
                        nc.tensor.transpose(
                            pt[:128, :tcnt],
                            xn8[:tcnt, cb * 128:(cb + 1) * 128],
                            ident[:tcnt, :tcnt])
                        dst = xT[cb // 2][:, cb % 2, e * EN + toff: e * EN + toff + tcnt]
                        if cb % 2 == 0:
                            nc.scalar.copy(dst, pt[:128, :tcnt])
                        else:
                            nc.vector.tensor_copy(dst, pt[:128, :tcnt])
                return xT

            for s in range(npairs):
                # ---------------- load x0 ----------------
                x0 = {}
                for e in range(2):
                    bidx = 2 * s + e
                    for j, (toff, tcnt) in enumerate(T_TILES):
                        t = resid.tile([128, C], FP32, tag=f"x0_{e}{j}")
                        nc.scalar.dma_start(t[:tcnt, :], x_d[bidx, toff:toff + tcnt, :])
                        x0[(e, j)] = t

                # ---------------- LN1 + fp8 transpose ----------------
                xnT = ln_transpose(x0, "ln1")

                # ---------------- q,k (weight-stationary DR) ----------------
                qkT = [act.tile([128, 2 * N], FP8, tag=f"qkT{b}") for b in range(12)]
                for b in range(12):
                    ps = ps_mm.tile([128, TT], FP32, tag="ps_mm")
                    for g in range(KG):
                        nc.tensor.matmul(
                            ps[:, :], wqk[g][:, :, b * 128:(b + 1) * 128],
                            xnT[g][:, :, :], start=(g == 0), stop=(g == KG - 1),
                            perf_mode=DR)
                    nc.vector.tensor_scalar(
                        qkT[b][:, :], ps[:, :], SQ / SW, qkb[:, b:b + 1],
                        op0=AL.mult, op1=AL.add)

                # ---------------- v (act-stationary DR) + vb row ----------------
                vt = {}
                for e in range(2):
                    v8 = act.tile([128, 2, H, 68], FP8, tag=f"vt{e}")
                    nc.gpsimd.memset(v8[:, :, :, :], 0.0)
                    nc.gpsimd.memset(v8[:, :, :, D:D + 1], SV)
                    vt[e] = v8
                for e in range(2):
                    for j, (toff, tcnt) in enumerate(T_TILES):
                        tp = T_PADS[j]
                        ts = e * EN + toff
                        for ci, (coff, csz) in enumerate(C_CHUNKS):
                            nh = csz // D
                            h0 = coff // D
                            ps = ps_mm.tile([128, 8, D], FP32, tag="ps_mm")
                            nc.tensor.matmul(
                                ps[:tp, :nh, :],
                                ones_x[:, :, :tp],
                                wvb[:, :, coff:coff + csz],
                                start=True, stop=False, perf_mode=DR)
                            for g in range(KG):
                                nc.tensor.matmul(
                                    ps[:tp, :nh, :],
                                    xnT[g][:, :, ts:ts + tp],
                                    wv[g][:, :, coff:coff + csz],
                                    start=False, stop=(g == KG - 1), perf_mode=DR)
                            nc.vector.tensor_scalar(
                                vt[e][:tcnt, j, h0:h0 + nh, 0:D],
                                ps[:tcnt, :nh, :], SV / SW, None, op0=AL.mult)

                # ---------------- attention ----------------
                aT = [act.tile([128, 2, 2 * N], FP8, tag=f"aT{g}") for g in range(KG)]
                for e in range(2):
                    for h in range(H):
                        qrow = 64 * (h % 2)
                        qt = qkT[h // 2]
                        kt = qkT[6 + h // 2]
                        et = expp.tile([128, 2, EN], FP8, tag="et")
                        nc.gpsimd.memset(et[64:, 1, :], 0.0)
                        nc.gpsimd.memset(et[:, :, N:], 0.0)
                        for j2, (tkoff, tkcnt) in enumerate(T_TILES):
                            tkp = T_PADS[j2]
                            L = ps_at.tile([128, EN], FP32, tag="ps_at")
                            nc.tensor.matmul(
                                L[:tkp, :EN],
                                kt[qrow:qrow + 64, e * EN + tkoff: e * EN + tkoff + tkp],
                                qt[qrow:qrow + 64, e * EN: e * EN + EN])
                            ex = expp.tile([128, N], BF16, tag="ex")
                            nc.scalar.activation(ex[:tkcnt, :], L[:tkcnt, :N],
                                                 AF.Exp, scale=1.0 / (SQ * SQ))
                            nc.gpsimd.tensor_tensor(
                                et[:tkcnt, j2, :N], ex[:tkcnt, :],
                                erpb[j2][:tkcnt, h, :], op=AL.mult)
                        O = ps_at.tile([68, EN], FP32, tag="ps_at")
                        nc.tensor.matmul(
                            O[:68, :EN],
                            vt[e][:, :, h, :],
                            et[:, :, :], perf_mode=DR)
                        lden = small.tile([1, N], FP32, tag="lden")
                        rb = small.tile([1, N], BF16, tag="recip")
                        nc.scalar.activation(lden[:, :], O[D:D + 1, :N], AF.Ln)
                        nc.scalar.activation(rb[:, :], lden[:, :], AF.Exp, scale=-1.0)
                        Dn = ps_at.tile([64, N], FP32, tag="ps_at")
                        nc.tensor.matmul(Dn[:, :], ones_bf[0:1, :], rb[0:1, :])
                        Dsb = expp.tile([64, N], BF16, tag="Dsb")
                        nc.scalar.copy(Dsb[:, :], Dn[:, :])
                        nc.vector.tensor_tensor(
                            aT[h // 4][64 * (h % 2):64 * (h % 2) + 64, (h % 4) // 2,
                                       e * EN: e * EN + N],
                            O[0:D, :N], Dsb[:, :], op=AL.mult)

                # ---------------- proj + residual -> x1 ----------------
                x1 = {}
                for e in range(2):
                    for j, (toff, tcnt) in enumerate(T_TILES):
                        tp = T_PADS[j]
                        xt = resid.tile([128, C], FP32, tag=f"x1_{e}{j}")
                        ts = e * EN + toff
                        for ci, (coff, csz) in enumerate(C_CHUNKS):
                            ps = ps_mm.tile([128, 512], FP32, tag="ps_mm")
                            for g in range(KG):
                                nc.tensor.matmul(
                                    ps[:tp, :csz],
                                    aT[g][:, :, ts:ts + tp],
                                    wp[g][:, :, coff:coff + csz],
                                    start=(g == 0), stop=(g == KG - 1), perf_mode=DR)
                            nc.vector.tensor_tensor(
                                xt[:tcnt, coff:coff + csz],
                                ps[:tcnt, :csz],
                                x0[(e, j)][:tcnt, coff:coff + csz], op=AL.add)
                        x1[(e, j)] = xt

                # ---------------- LN2 + fp8 transpose ----------------
                hnT = ln_transpose(x1, "ln2")

                # ---------------- fc1 + gelu -> hT ----------------
                hT = [act.tile([128, 2, 2 * N], FP8, tag=f"hT{g}") for g in range(KG2)]
                for ob in range(24):
                    ps = ps_mm.tile([128, TT], FP32, tag="ps_mm")
                    for g in range(KG):
                        nc.tensor.matmul(
                            ps[:, :], wf1[g][:, :, ob * 128:(ob + 1) * 128],
                            hnT[g][:, :, :], start=(g == 0), stop=(g == KG - 1),
                            perf_mode=DR)
                    nc.scalar.activation(
                        hT[ob // 2][:, ob % 2, :], ps[:, :], AF.Gelu,
                        scale=1.0 / SW, bias=f1b[:, ob:ob + 1])

                # ---------------- fc2 + residual -> y ----------------
                for e in range(2):
                    bidx = 2 * s + e
                    for j, (toff, tcnt) in enumerate(T_TILES):
                        tp = T_PADS[j]
                        ot = resid.tile([128, C], FP32, tag=f"out_{e}{j}")
                        ts = e * EN + toff
                        for ci, (coff, csz) in enumerate(C_CHUNKS):
                            ps = ps_mm.tile([128, 512], FP32, tag="ps_mm")
                            for g in range(KG2):
                                nc.tensor.matmul(
                                    ps[:tp, :csz],
                                    hT[g][:, :, ts:ts + tp],
                                    wf2[g][:, :, coff:coff + csz],
                                    start=(g == 0), stop=(g == KG2 - 1), perf_mode=DR)
                            nc.vector.tensor_tensor(
                                ot[:tcnt, coff:coff + csz],
                                ps[:tcnt, :csz],
                                x1[(e, j)][:tcnt, coff:coff + csz], op=AL.add)
                        nc.gpsimd.dma_start(y_d[bidx, toff:toff + tcnt, :], ot[:tcnt, :])

    return nc


def fold_weights(inputs):
    """Host-side folding into fp8 DoubleRow layouts. Returns per-core dict."""
    import ml_dtypes
    f8 = ml_dtypes.float8_e4m3
    bf = ml_dtypes.bfloat16
    f32 = np.float32
    g = {k: np.asarray(v) for k, v in inputs.items()}
    n1w, n1b = g["n1_w"].astype(f32), g["n1_b"].astype(f32)
    n2w, n2b = g["n2_w"].astype(f32), g["n2_b"].astype(f32)
    g1, g2 = g["gamma1"].astype(f32), g["gamma2"].astype(f32)
    qkv_w = g["qkv_w"].astype(f32)
    q_bias, v_bias = g["q_bias"].astype(f32), g["v_bias"].astype(f32)
    proj_w, proj_b = g["proj_w"].astype(f32), g["proj_b"].astype(f32)
    fc1_w, fc1_b = g["fc1_w"].astype(f32), g["fc1_b"].astype(f32)
    fc2_w, fc2_b = g["fc2_w"].astype(f32), g["fc2_b"].astype(f32)
    assert np.all(proj_b == 0), "kernel assumes proj_b == 0"

    qkv_bias = np.concatenate([q_bias, np.zeros_like(q_bias), v_bias])
    Wq = qkv_w * n1w[None, :]                       # LN affine fold
    bq = qkv_bias + qkv_w @ n1b
    scale = D ** -0.5
    Wq[:C] *= scale
    bq[:C] *= scale

    def dr_pack(wT, ngroups):
        # wT: [in_features, out] -> [ngroups, 128, 2, out]
        nin = wT.shape[0]
        assert nin == ngroups * 256
        return np.ascontiguousarray(
            wT.reshape(ngroups, 2, 128, -1).transpose(0, 2, 1, 3))

    wqk = dr_pack((SW * Wq[:2 * C].T), KG).astype(f8)
    wv = dr_pack((SW * Wq[2 * C:].T), KG).astype(f8)
    wvb = np.zeros((1, 2, C), np.float32)
    wvb[0, 0, :] = SW * bq[2 * C:]
    Pw = (g1[:, None] * proj_w)
    wp = dr_pack((R * Pw.T), KG).astype(f8)
    F1 = fc1_w * n2w[None, :]
    f1b_full = fc1_b + fc1_w @ n2b
    wf1 = dr_pack((SW * F1.T), KG).astype(f8)
    F2 = g2[:, None] * fc2_w
    wf2 = dr_pack((R * F2.T), KG2).astype(f8)

    qkb = (SQ * bq[:2 * C]).reshape(12, 128).T.copy()
    f1b = f1b_full.reshape(24, 128).T.copy()

    table = g["rel_bias_table"].astype(f32)
    idx = np.asarray(g["rel_index"]).reshape(-1)
    rpb = table[idx].reshape(N, N, H).transpose(2, 0, 1)   # [h, tq, tk]
    rpbT = rpb.transpose(0, 2, 1)                          # [h, tk, tq]
    erpb = np.exp(rpbT)
    erpb0 = np.ascontiguousarray(erpb[:, :128, :].transpose(1, 0, 2)).astype(bf)
    erpb1 = np.ascontiguousarray(erpb[:, 128:, :].transpose(1, 0, 2)).astype(bf)

    return {
        "wqk": wqk, "wv": wv, "wvb": wvb.astype(f8), "wp": wp,
        "wf1": wf1, "wf2": wf2,
        "qkb": np.ascontiguousarray(qkb), "f1b": np.ascontiguousarray(f1b),
        "erpb0": erpb0, "erpb1": erpb1,
    }, (g2 * fc2_b).astype(f32)


_CACHE = {}


def _get_nc():
    if "nc" not in _CACHE:
        nc = build_nc()
        patched = _legalize_waits(nc.to_json_bytes())
        nc.to_json_bytes = lambda: patched
        _CACHE["nc"] = nc
    return _CACHE["nc"]


def kernel(**inputs):
    from concourse.bass_utils import run_bass_kernel_spmd
    nc = _get_nc()
    folded, f2b_host = fold_weights(inputs)
    x = np.ascontiguousarray(np.asarray(inputs["x"], dtype=np.float32))
    assert x.shape == (B, N, C), x.shape
    xs = R * x
    in_maps = []
    for c in range(NCORES):
        m = dict(folded)
        m["x"] = np.ascontiguousarray(xs[c * BPC:(c + 1) * BPC])
        in_maps.append(m)
    res = run_bass_kernel_spmd(nc, in_maps, core_ids=list(range(NCORES)))
    out = np.concatenate([res.results[c]["y"] for c in range(NCORES)], axis=0)
    return (out * (1.0 / R) + f2b_host).astype(np.float32)


# revision 14
# speedup vs baseline: 1.3819x; 1.0170x over previous
"""Trainium2 Bass kernel for nn_Block_74363063763569 (BEiT-style transformer block).

Data-parallel over batch across 8 NeuronCores (8 elems/core), zero collectives.
fp8e4m3 DoubleRow GEMMs; see build_nc docstring for the numerics scheme.
"""
import sys, json
sys.path.insert(0, "/opt/trn_rl_repo")
import numpy as np


def _legalize_waits(bir_bytes, max_waits=1):
    """This container's walrus rejects >1 sync wait per instruction; split
    extras into preceding single-wait EventSemaphore instructions."""
    j = json.loads(bir_bytes)
    for f in j["functions"]:
        for b in f["blocks"]:
            out = []
            for inst in b["instructions"]:
                si = inst.get("sync_info")
                waits = si.get("on_wait", []) if si else []
                if len(waits) > max_waits:
                    keep, extra = waits[:max_waits], waits[max_waits:]
                    for k, w in enumerate(extra):
                        out.append({"debug": inst.get("debug", 0), "engine": inst["engine"],
                                    "ins": [], "name": f"{inst['name']}_w{k}",
                                    "opcode": "EventSemaphore", "outs": [],
                                    "sync_info": {"on_update": [], "on_wait": [w]}})
                    si["on_wait"] = keep
                out.append(inst)
            b["instructions"] = out
    return json.dumps(j).encode()


import concourse.bass as bass
import concourse.tile as tile
import concourse.mybir as mybir
from concourse.masks import make_identity

FP32 = mybir.dt.float32
BF16 = mybir.dt.bfloat16
FP8 = mybir.dt.float8e4
DR = mybir.MatmulPerfMode.DoubleRow

B = 64
N = 197
C = 768
H = 12
D = 64
HID = 3072
NCORES = 8
BPC = B // NCORES
NPAIRS_FULL = BPC // 2
KG = C // 256      # 3 doublerow contraction groups over C
KG2 = HID // 256   # 12 groups over HID
LN_EPS = 1e-5

R = 256.0    # residual stream scale
SW = 64.0    # qkv/fc1 weight scale
SQ = 8.0     # q/k fp8 activation scale
SV = 8.0     # v fp8 activation scale (ones col = SV)

T_TILES = [(0, 128), (128, 69)]
T_PADS = [128, 72]           # padded token counts for fp8 stationary slices
EN = 200                     # padded per-elem token stride (4-aligned offsets)
TT = 2 * EN                  # packed token extent
C_CHUNKS = [(0, 512), (512, 256)]

AL = mybir.AluOpType
AF = mybir.ActivationFunctionType


def build_nc(npairs=NPAIRS_FULL):
    nb = 2 * npairs
    nc = bass.Bass()

    x_d = nc.dram_tensor("x", [nb, N, C], FP32, kind="ExternalInput")
    wqk_d = nc.dram_tensor("wqk", [KG, 128, 2, 1536], FP8, kind="ExternalInput")
    wv_d = nc.dram_tensor("wv", [KG, 128, 2, C], FP8, kind="ExternalInput")
    wvb_d = nc.dram_tensor("wvb", [1, 2, C], FP8, kind="ExternalInput")
    wp_d = nc.dram_tensor("wp", [KG, 128, 2, C], FP8, kind="ExternalInput")
    wf1_d = nc.dram_tensor("wf1", [KG, 128, 2, HID], FP8, kind="ExternalInput")
    wf2_d = nc.dram_tensor("wf2", [KG2, 128, 2, C], FP8, kind="ExternalInput")
    qkb_d = nc.dram_tensor("qkb", [128, 12], FP32, kind="ExternalInput")
    f1b_d = nc.dram_tensor("f1b", [128, 24], FP32, kind="ExternalInput")
    rpb0_d = nc.dram_tensor("rpb0", [128, H, EN], FP8, kind="ExternalInput")
    rpb1_d = nc.dram_tensor("rpb1", [72, H, EN], FP8, kind="ExternalInput")
    y_d = nc.dram_tensor("y", [nb, N, C], FP32, kind="ExternalOutput")

    with tile.TileContext(nc) as tc:
        with (
            tc.tile_pool(name="singles", bufs=1) as singles,
            tc.tile_pool(name="resid", bufs=2) as resid,
            tc.tile_pool(name="act", bufs=2) as act,      # per-pair fp8 activations
            tc.tile_pool(name="xn", bufs=3) as xnp,
            tc.tile_pool(name="expp", bufs=4) as expp,
            tc.tile_pool(name="small", bufs=8) as small,
            tc.tile_pool(name="ps_tr", bufs=2, space="PSUM") as ps_tr,
            tc.tile_pool(name="ps_mm", bufs=2, space="PSUM") as ps_mm,
            tc.tile_pool(name="ps_at", bufs=4, space="PSUM") as ps_at,
        ):
            # ---- persistent weights / constants ----
            wqk = [singles.tile([128, 2, 1536], FP8, tag=f"wqk{g}") for g in range(KG)]
            wv = [singles.tile([128, 2, C], FP8, tag=f"wv{g}") for g in range(KG)]
            wvb = singles.tile([1, 2, C], FP8, tag="wvb")
            wp = [singles.tile([128, 2, C], FP8, tag=f"wp{g}") for g in range(KG)]
            wf1 = [singles.tile([128, 2, HID], FP8, tag=f"wf1{g}") for g in range(KG)]
            wf2 = [singles.tile([128, 2, C], FP8, tag=f"wf2{g}") for g in range(KG2)]
            qkb = singles.tile([128, 12], FP32, tag="qkb")
            f1b = singles.tile([128, 24], FP32, tag="f1b")
            erpb = [singles.tile([128, H, N], BF16, tag="erpb0"),
                    singles.tile([69, H, N], BF16, tag="erpb1")]
            ident = singles.tile([128, 128], BF16, tag="ident")
            ones_x = singles.tile([1, 2, 128], FP8, tag="ones_x")
            ones_bf = singles.tile([1, 64], BF16, tag="ones_bf")
            eps_sb = singles.tile([128, 1], FP32, tag="eps")

            for g in range(KG):
                nc.sync.dma_start(wqk[g][:], wqk_d[g])
                nc.sync.dma_start(wv[g][:], wv_d[g])
                nc.sync.dma_start(wp[g][:], wp_d[g])
                nc.sync.dma_start(wf1[g][:], wf1_d[g])
            for g in range(KG2):
                nc.sync.dma_start(wf2[g][:], wf2_d[g])
            nc.sync.dma_start(wvb[:], wvb_d[:])
            nc.sync.dma_start(qkb[:], qkb_d[:])
            nc.sync.dma_start(f1b[:], f1b_d[:])
            nc.sync.dma_start(rpb8[0][:], rpb0_d[:])
            nc.sync.dma_start(rpb8[1][:], rpb1_d[:])
            make_identity(nc, ident[:])
            make_identity(nc, ident8[:])
            nc.vector.memset(ones_x[:], 1.0)
            nc.vector.memset(ones_bf[:], 1.0)
            nc.vector.memset(eps_sb[:], LN_EPS)

            def ln_transpose(x_tiles, tag):
                """LN over features + fp8 transpose into [128, 2, 2N] group tiles."""
                xT = [act.tile([128, 2, 2 * N], FP8, tag=f"{tag}T{g}") for g in range(KG)]
                for (e, j), xt in x_tiles.items():
                    toff, tcnt = T_TILES[j]
                    stats = small.tile([128, 3, 6], FP32, tag=f"st_{tag}")
                    mv = small.tile([128, 2], FP32, tag=f"mv_{tag}")
                    sd = small.tile([128, 1], FP32, tag=f"sd_{tag}")
                    rstd = small.tile([128, 1], FP32, tag=f"rs_{tag}")
                    for g3 in range(3):
                        nc.vector.bn_stats(stats[:tcnt, g3, :], xt[:tcnt, g3 * 256:(g3 + 1) * 256])
                    nc.vector.bn_aggr(mv[:tcnt], stats[:tcnt])
                    nc.scalar.activation(sd[:tcnt], mv[:tcnt, 1:2], AF.Ln, bias=eps_sb[:tcnt])
                    nc.scalar.activation(rstd[:tcnt], sd[:tcnt], AF.Exp, scale=-0.5)
                    xn8 = xnp.tile([128, C], BF16, tag="xn8")
                    nc.vector.tensor_scalar(
                        xn8[:tcnt, :], xt[:tcnt, :],
                        mv[:tcnt, 0:1], rstd[:tcnt, 0:1],
                        op0=AL.subtract, op1=AL.mult)
                    for cb in range(6):
                        pt = ps_tr.tile([128, 128], BF16, tag="ps_tr")
                        nc.tensor.transpose(
                            pt[:128, :tcnt],
                            xn8[:tcnt, cb * 128:(cb + 1) * 128],
                            ident[:tcnt, :tcnt])
                        dst = xT[cb // 2][:, cb % 2, e * EN + toff: e * EN + toff + tcnt]
                        if cb % 2 == 0:
                            nc.scalar.copy(dst, pt[:128, :tcnt])
                        else:
                            nc.vector.tensor_copy(dst, pt[:128, :tcnt])
                return xT

            for s in range(npairs):
                # ---------------- load x0 ----------------
                x0 = {}
                for e in range(2):
                    bidx = 2 * s + e
                    for j, (toff, tcnt) in enumerate(T_TILES):
                        t = resid.tile([128, C], FP32, tag=f"x0_{e}{j}")
                        nc.scalar.dma_start(t[:tcnt, :], x_d[bidx, toff:toff + tcnt, :])
                        x0[(e, j)] = t

                # ---------------- LN1 + fp8 transpose ----------------
                xnT = ln_transpose(x0, "ln1")

                # ---------------- q,k (weight-stationary DR) ----------------
                qkT = [act.tile([128, 2 * N], FP8, tag=f"qkT{b}") for b in range(12)]
                for b in range(12):
                    ps = ps_mm.tile([128, TT], FP32, tag="ps_mm")
                    for g in range(KG):
                        nc.tensor.matmul(
                            ps[:, :], wqk[g][:, :, b * 128:(b + 1) * 128],
                            xnT[g][:, :, :], start=(g == 0), stop=(g == KG - 1),
                            perf_mode=DR)
                    nc.vector.tensor_scalar(
                        qkT[b][:, :], ps[:, :], SQ / SW, qkb[:, b:b + 1],
                        op0=AL.mult, op1=AL.add)

                # ---------------- v (act-stationary DR) + vb row ----------------
                vt = {}
                for e in range(2):
                    v8 = act.tile([128, 2, H, 68], FP8, tag=f"vt{e}")
                    nc.gpsimd.memset(v8[:, :, :, :], 0.0)
                    nc.gpsimd.memset(v8[:, :, :, D:D + 1], SV)
                    vt[e] = v8
                for e in range(2):
                    for j, (toff, tcnt) in enumerate(T_TILES):
                        tp = T_PADS[j]
                        ts = e * EN + toff
                        for ci, (coff, csz) in enumerate(C_CHUNKS):
                            nh = csz // D
                            h0 = coff // D
                            ps = ps_mm.tile([128, 8, D], FP32, tag="ps_mm")
                            nc.tensor.matmul(
                                ps[:tp, :nh, :],
                                ones_x[:, :, :tp],
                                wvb[:, :, coff:coff + csz],
                                start=True, stop=False, perf_mode=DR)
                            for g in range(KG):
                                nc.tensor.matmul(
                                    ps[:tp, :nh, :],
                                    xnT[g][:, :, ts:ts + tp],
                                    wv[g][:, :, coff:coff + csz],
                                    start=False, stop=(g == KG - 1), perf_mode=DR)
                            nc.vector.tensor_scalar(
                                vt[e][:tcnt, j, h0:h0 + nh, 0:D],
                                ps[:tcnt, :nh, :], SV / SW, None, op0=AL.mult)

                # ---------------- attention ----------------
                aT = [act.tile([128, 2, 2 * N], FP8, tag=f"aT{g}") for g in range(KG)]
                for e in range(2):
                    for h in range(H):
                        qrow = 64 * (h % 2)
                        qt = qkT[h // 2]
                        kt = qkT[6 + h // 2]
                        et = expp.tile([128, 2, EN], FP8, tag="et")
                        nc.gpsimd.memset(et[64:, 1, :], 0.0)
                        nc.gpsimd.memset(et[:, :, N:], 0.0)
                        for j2, (tkoff, tkcnt) in enumerate(T_TILES):
                            tkp = T_PADS[j2]
                            L = ps_at.tile([128, EN], FP32, tag="ps_at")
                            nc.tensor.matmul(
                                L[:tkp, :EN],
                                kt[qrow:qrow + 64, e * EN + tkoff: e * EN + tkoff + tkp],
                                qt[qrow:qrow + 64, e * EN: e * EN + EN],
                                start=True, stop=False)
                            nc.tensor.matmul(
                                L[:tkp, :EN],
                                ident8[:tkp, :tkp],
                                rpb8[j2][:tkp, h, :],
                                start=False, stop=True)
                            nc.scalar.activation(et[:tkcnt, j2, :N], L[:tkcnt, :N],
                                                 AF.Exp, scale=1.0 / (SQ * SQ))
                        O = ps_at.tile([68, EN], FP32, tag="ps_at")
                        nc.tensor.matmul(
                            O[:68, :EN],
                            vt[e][:, :, h, :],
                            et[:, :, :], perf_mode=DR)
                        lden = small.tile([1, N], FP32, tag="lden")
                        rb = small.tile([1, N], BF16, tag="recip")
                        nc.scalar.activation(lden[:, :], O[D:D + 1, :N], AF.Ln)
                        nc.scalar.activation(rb[:, :], lden[:, :], AF.Exp, scale=-1.0)
                        Dn = ps_at.tile([64, N], FP32, tag="ps_at")
                        nc.tensor.matmul(Dn[:, :], ones_bf[0:1, :], rb[0:1, :])
                        Dsb = expp.tile([64, N], BF16, tag="Dsb")
                        nc.scalar.copy(Dsb[:, :], Dn[:, :])
                        nc.vector.tensor_tensor(
                            aT[h // 4][64 * (h % 2):64 * (h % 2) + 64, (h % 4) // 2,
                                       e * EN: e * EN + N],
                            O[0:D, :N], Dsb[:, :], op=AL.mult)

                # ---------------- proj + residual -> x1 ----------------
                x1 = {}
                for e in range(2):
                    for j, (toff, tcnt) in enumerate(T_TILES):
                        tp = T_PADS[j]
                        xt = resid.tile([128, C], FP32, tag=f"x1_{e}{j}")
                        ts = e * EN + toff
                        for ci, (coff, csz) in enumerate(C_CHUNKS):
                            ps = ps_mm.tile([128, 512], FP32, tag="ps_mm")
                            for g in range(KG):
                                nc.tensor.matmul(
                                    ps[:tp, :csz],
                                    aT[g][:, :, ts:ts + tp],
                                    wp[g][:, :, coff:coff + csz],
                                    start=(g == 0), stop=(g == KG - 1), perf_mode=DR)
                            nc.vector.tensor_tensor(
                                xt[:tcnt, coff:coff + csz],
                                ps[:tcnt, :csz],
                                x0[(e, j)][:tcnt, coff:coff + csz], op=AL.add)
                        x1[(e, j)] = xt

                # ---------------- LN2 + fp8 transpose ----------------
                hnT = ln_transpose(x1, "ln2")

                # ---------------- fc1 + gelu -> hT ----------------
                hT = [act.tile([128, 2, 2 * N], FP8, tag=f"hT{g}") for g in range(KG2)]
                for ob in range(24):
                    ps = ps_mm.tile([128, TT], FP32, tag="ps_mm")
                    for g in range(KG):
                        nc.tensor.matmul(
                            ps[:, :], wf1[g][:, :, ob * 128:(ob + 1) * 128],
                            hnT[g][:, :, :], start=(g == 0), stop=(g == KG - 1),
                            perf_mode=DR)
                    nc.scalar.activation(
                        hT[ob // 2][:, ob % 2, :], ps[:, :], AF.Gelu,
                        scale=1.0 / SW, bias=f1b[:, ob:ob + 1])

                # ---------------- fc2 + residual -> y ----------------
                for e in range(2):
                    bidx = 2 * s + e
                    for j, (toff, tcnt) in enumerate(T_TILES):
                        tp = T_PADS[j]
                        ot = resid.tile([128, C], FP32, tag=f"x0_{e}{j}", name=f"out_{e}{j}")
                        ts = e * EN + toff
                        for ci, (coff, csz) in enumerate(C_CHUNKS):
                            ps = ps_mm.tile([128, 512], FP32, tag="ps_mm")
                            for g in range(KG2):
                                nc.tensor.matmul(
                                    ps[:tp, :csz],
                                    hT[g][:, :, ts:ts + tp],
                                    wf2[g][:, :, coff:coff + csz],
                                    start=(g == 0), stop=(g == KG2 - 1), perf_mode=DR)
                            nc.vector.tensor_tensor(
                                ot[:tcnt, coff:coff + csz],
                                ps[:tcnt, :csz],
                                x1[(e, j)][:tcnt, coff:coff + csz], op=AL.add)
                        nc.gpsimd.dma_start(y_d[bidx, toff:toff + tcnt, :], ot[:tcnt, :])

    return nc


def fold_weights(inputs):
    """Host-side folding into fp8 DoubleRow layouts. Returns per-core dict."""
    import ml_dtypes
    f8 = ml_dtypes.float8_e4m3
    bf = ml_dtypes.bfloat16
    f32 = np.float32
    g = {k: np.asarray(v) for k, v in inputs.items()}
    n1w, n1b = g["n1_w"].astype(f32), g["n1_b"].astype(f32)
    n2w, n2b = g["n2_w"].astype(f32), g["n2_b"].astype(f32)
    g1, g2 = g["gamma1"].astype(f32), g["gamma2"].astype(f32)
    qkv_w = g["qkv_w"].astype(f32)
    q_bias, v_bias = g["q_bias"].astype(f32), g["v_bias"].astype(f32)
    proj_w, proj_b = g["proj_w"].astype(f32), g["proj_b"].astype(f32)
    fc1_w, fc1_b = g["fc1_w"].astype(f32), g["fc1_b"].astype(f32)
    fc2_w, fc2_b = g["fc2_w"].astype(f32), g["fc2_b"].astype(f32)
    assert np.all(proj_b == 0), "kernel assumes proj_b == 0"

    qkv_bias = np.concatenate([q_bias, np.zeros_like(q_bias), v_bias])
    Wq = qkv_w * n1w[None, :]                       # LN affine fold
    bq = qkv_bias + qkv_w @ n1b
    scale = D ** -0.5
    Wq[:C] *= scale
    bq[:C] *= scale

    def dr_pack(wT, ngroups):
        # wT: [in_features, out] -> [ngroups, 128, 2, out]
        nin = wT.shape[0]
        assert nin == ngroups * 256
        return np.ascontiguousarray(
            wT.reshape(ngroups, 2, 128, -1).transpose(0, 2, 1, 3))

    wqk = dr_pack((SW * Wq[:2 * C].T), KG).astype(f8)
    wv = dr_pack((SW * Wq[2 * C:].T), KG).astype(f8)
    wvb = np.zeros((1, 2, C), np.float32)
    wvb[0, 0, :] = SW * bq[2 * C:]
    Pw = (g1[:, None] * proj_w)
    wp = dr_pack((R * Pw.T), KG).astype(f8)
    F1 = fc1_w * n2w[None, :]
    f1b_full = fc1_b + fc1_w @ n2b
    wf1 = dr_pack((SW * F1.T), KG).astype(f8)
    F2 = g2[:, None] * fc2_w
    wf2 = dr_pack((R * F2.T), KG2).astype(f8)

    qkb = (SQ * bq[:2 * C]).reshape(12, 128).T.copy()
    f1b = f1b_full.reshape(24, 128).T.copy()

    table = g["rel_bias_table"].astype(f32)
    idx = np.asarray(g["rel_index"]).reshape(-1)
    rpb = table[idx].reshape(N, N, H).transpose(2, 0, 1)   # [h, tq, tk]
    rpbT = rpb.transpose(0, 2, 1)                          # [h, tk, tq]
    rpb0 = np.zeros((128, H, 200), np.float32)
    rpb1 = np.zeros((72, H, 200), np.float32)
    rpb0[:, :, :N] = (SQ * SQ) * rpbT[:, :128, :].transpose(1, 0, 2)
    rpb1[:69, :, :N] = (SQ * SQ) * rpbT[:, 128:, :].transpose(1, 0, 2)

    return {
        "wqk": wqk, "wv": wv, "wvb": wvb.astype(f8), "wp": wp,
        "wf1": wf1, "wf2": wf2,
        "qkb": np.ascontiguousarray(qkb), "f1b": np.ascontiguousarray(f1b),
        "rpb0": rpb0.astype(f8), "rpb1": rpb1.astype(f8),
    }, (g2 * fc2_b).astype(f32)


_CACHE = {}


def _get_nc():
    if "nc" not in _CACHE:
        nc = build_nc()
        patched = _legalize_waits(nc.to_json_bytes())
        nc.to_json_bytes = lambda: patched
        _CACHE["nc"] = nc
    return _CACHE["nc"]


def kernel(**inputs):
    from concourse.bass_utils import run_bass_kernel_spmd
    nc = _get_nc()
    folded, f2b_host = fold_weights(inputs)
    x = np.ascontiguousarray(np.asarray(inputs["x"], dtype=np.float32))
    assert x.shape == (B, N, C), x.shape
    xs = R * x
    in_maps = []
    for c in range(NCORES):
        m = dict(folded)
        m["x"] = np.ascontiguousarray(xs[c * BPC:(c + 1) * BPC])
        in_maps.append(m)
    res = run_bass_kernel_spmd(nc, in_maps, core_ids=list(range(NCORES)))
    out = np.concatenate([res.results[c]["y"] for c in range(NCORES)], axis=0)
    return (out * (1.0 / R) + f2b_host).astype(np.float32)


# revision 15
# speedup vs baseline: 1.6437x; 1.1895x over previous
"""Trainium2 Bass kernel for nn_Block_74363063763569 (BEiT-style transformer block).

Data-parallel over batch across 8 NeuronCores (8 elems/core), zero collectives.
fp8e4m3 DoubleRow GEMMs; see build_nc docstring for the numerics scheme.
"""
import sys, json
sys.path.insert(0, "/opt/trn_rl_repo")
import numpy as np


def _legalize_waits(bir_bytes, max_waits=1):
    """This container's walrus rejects >1 sync wait per instruction; split
    extras into preceding single-wait EventSemaphore instructions."""
    j = json.loads(bir_bytes)
    for f in j["functions"]:
        for b in f["blocks"]:
            out = []
            for inst in b["instructions"]:
                si = inst.get("sync_info")
                waits = si.get("on_wait", []) if si else []
                if len(waits) > max_waits:
                    keep, extra = waits[:max_waits], waits[max_waits:]
                    for k, w in enumerate(extra):
                        out.append({"debug": inst.get("debug", 0), "engine": inst["engine"],
                                    "ins": [], "name": f"{inst['name']}_w{k}",
                                    "opcode": "EventSemaphore", "outs": [],
                                    "sync_info": {"on_update": [], "on_wait": [w]}})
                    si["on_wait"] = keep
                out.append(inst)
            b["instructions"] = out
    return json.dumps(j).encode()


import concourse.bass as bass
import concourse.tile as tile
import concourse.mybir as mybir
from concourse.masks import make_identity

FP32 = mybir.dt.float32
BF16 = mybir.dt.bfloat16
FP8 = mybir.dt.float8e4
DR = mybir.MatmulPerfMode.DoubleRow

B = 64
N = 197
C = 768
H = 12
D = 64
HID = 3072
NCORES = 8
BPC = B // NCORES
NPAIRS_FULL = BPC // 2
KG = C // 256      # 3 doublerow contraction groups over C
KG2 = HID // 256   # 12 groups over HID
LN_EPS = 1e-5

R = 256.0    # residual stream scale
SW = 64.0    # qkv/fc1 weight scale
SQ = 8.0     # q/k fp8 activation scale
SV = 8.0     # v fp8 activation scale (ones col = SV)

T_TILES = [(0, 128), (128, 69)]
T_PADS = [128, 72]           # padded token counts for fp8 stationary slices
EN = 200                     # padded per-elem token stride (4-aligned offsets)
TT = 2 * EN                  # packed token extent
C_CHUNKS = [(0, 512), (512, 256)]

AL = mybir.AluOpType
AF = mybir.ActivationFunctionType


def build_nc(npairs=NPAIRS_FULL):
    nb = 2 * npairs
    nc = bass.Bass()

    x_d = nc.dram_tensor("x", [nb, N, C], FP32, kind="ExternalInput")
    wqk_d = nc.dram_tensor("wqk", [KG, 128, 2, 1536], FP8, kind="ExternalInput")
    wv_d = nc.dram_tensor("wv", [KG, 128, 2, C], FP8, kind="ExternalInput")
    wvb_d = nc.dram_tensor("wvb", [1, 2, C], FP8, kind="ExternalInput")
    wp_d = nc.dram_tensor("wp", [KG, 128, 2, C], FP8, kind="ExternalInput")
    wf1_d = nc.dram_tensor("wf1", [KG, 128, 2, HID], FP8, kind="ExternalInput")
    wf2_d = nc.dram_tensor("wf2", [KG2, 128, 2, C], FP8, kind="ExternalInput")
    qkb_d = nc.dram_tensor("qkb", [128, 12], FP32, kind="ExternalInput")
    f1b_d = nc.dram_tensor("f1b", [128, 24], FP32, kind="ExternalInput")
    rpb0_d = nc.dram_tensor("rpb0", [128, H, EN], FP8, kind="ExternalInput")
    rpb1_d = nc.dram_tensor("rpb1", [72, H, EN], FP8, kind="ExternalInput")
    y_d = nc.dram_tensor("y", [nb, N, C], FP32, kind="ExternalOutput")

    with tile.TileContext(nc) as tc:
        with (
            tc.tile_pool(name="singles", bufs=1) as singles,
            tc.tile_pool(name="resid", bufs=2) as resid,
            tc.tile_pool(name="act", bufs=2) as act,      # per-pair fp8 activations
            tc.tile_pool(name="xn", bufs=3) as xnp,
            tc.tile_pool(name="expp", bufs=4) as expp,
            tc.tile_pool(name="small", bufs=3) as small,
            tc.tile_pool(name="ps_tr", bufs=2, space="PSUM") as ps_tr,
            tc.tile_pool(name="ps_mm", bufs=2, space="PSUM") as ps_mm,
            tc.tile_pool(name="ps_at", bufs=4, space="PSUM") as ps_at,
        ):
            # ---- persistent weights / constants ----
            wqk = [singles.tile([128, 2, 1536], FP8, tag=f"wqk{g}") for g in range(KG)]
            wv = [singles.tile([128, 2, C], FP8, tag=f"wv{g}") for g in range(KG)]
            wvb = singles.tile([1, 2, C], FP8, tag="wvb")
            wp = [singles.tile([128, 2, C], FP8, tag=f"wp{g}") for g in range(KG)]
            wf1 = [singles.tile([128, 2, HID], FP8, tag=f"wf1{g}") for g in range(KG)]
            wf2 = [singles.tile([128, 2, C], FP8, tag=f"wf2{g}") for g in range(KG2)]
            qkb = singles.tile([128, 12], FP32, tag="qkb")
            f1b = singles.tile([128, 24], FP32, tag="f1b")
            erpb = [singles.tile([128, H, N], BF16, tag="erpb0"),
                    singles.tile([69, H, N], BF16, tag="erpb1")]
            ident = singles.tile([128, 128], BF16, tag="ident")
            ones_x = singles.tile([1, 2, 128], FP8, tag="ones_x")
            ones_bf = singles.tile([1, 64], BF16, tag="ones_bf")
            eps_sb = singles.tile([128, 1], FP32, tag="eps")

            for g in range(KG):
                nc.sync.dma_start(wqk[g][:], wqk_d[g])
                nc.sync.dma_start(wv[g][:], wv_d[g])
                nc.sync.dma_start(wp[g][:], wp_d[g])
                nc.sync.dma_start(wf1[g][:], wf1_d[g])
            for g in range(KG2):
                nc.sync.dma_start(wf2[g][:], wf2_d[g])
            nc.sync.dma_start(wvb[:], wvb_d[:])
            nc.sync.dma_start(qkb[:], qkb_d[:])
            nc.sync.dma_start(f1b[:], f1b_d[:])
            nc.sync.dma_start(rpb8[0][:], rpb0_d[:])
            nc.sync.dma_start(rpb8[1][:], rpb1_d[:])
            make_identity(nc, ident[:])
            make_identity(nc, ident8[:])
            nc.vector.memset(ones_x[:], 1.0)
            nc.vector.memset(ones_bf[:], 1.0)
            nc.vector.memset(eps_sb[:], LN_EPS)

            def ln_transpose(x_tiles, tag):
                """LN over features + fp8 transpose into [128, 2, 2N] group tiles."""
                xT = [act.tile([128, 2, 2 * N], FP8, tag=f"{tag}T{g}") for g in range(KG)]
                for (e, j), xt in x_tiles.items():
                    toff, tcnt = T_TILES[j]
                    stats = small.tile([128, 3, 6], FP32, tag=f"st_{tag}")
                    mv = small.tile([128, 2], FP32, tag=f"mv_{tag}")
                    sd = small.tile([128, 1], FP32, tag=f"sd_{tag}")
                    rstd = small.tile([128, 1], FP32, tag=f"rs_{tag}")
                    for g3 in range(3):
                        nc.vector.bn_stats(stats[:tcnt, g3, :], xt[:tcnt, g3 * 256:(g3 + 1) * 256])
                    nc.vector.bn_aggr(mv[:tcnt], stats[:tcnt])
                    nc.scalar.activation(sd[:tcnt], mv[:tcnt, 1:2], AF.Ln, bias=eps_sb[:tcnt])
                    nc.scalar.activation(rstd[:tcnt], sd[:tcnt], AF.Exp, scale=-0.5)
                    xn8 = xnp.tile([128, C], BF16, tag="xn8")
                    nc.vector.tensor_scalar(
                        xn8[:tcnt, :], xt[:tcnt, :],
                        mv[:tcnt, 0:1], rstd[:tcnt, 0:1],
                        op0=AL.subtract, op1=AL.mult)
                    for cb in range(6):
                        pt = ps_tr.tile([128, 128], BF16, tag="ps_tr")
                        nc.tensor.transpose(
                            pt[:128, :tcnt],
                            xn8[:tcnt, cb * 128:(cb + 1) * 128],
                            ident[:tcnt, :tcnt])
                        dst = xT[cb // 2][:, cb % 2, e * EN + toff: e * EN + toff + tcnt]
                        if cb % 2 == 0:
                            nc.scalar.copy(dst, pt[:128, :tcnt])
                        else:
                            nc.vector.tensor_copy(dst, pt[:128, :tcnt])
                return xT

            for s in range(npairs):
                # ---------------- load x0 ----------------
                x0 = {}
                for e in range(2):
                    bidx = 2 * s + e
                    for j, (toff, tcnt) in enumerate(T_TILES):
                        t = resid.tile([128, C], FP32, tag=f"x0_{e}{j}")
                        nc.scalar.dma_start(t[:tcnt, :], x_d[bidx, toff:toff + tcnt, :])
                        x0[(e, j)] = t

                # ---------------- LN1 + fp8 transpose ----------------
                xnT = ln_transpose(x0, "ln1")

                # ---------------- q,k (weight-stationary DR) ----------------
                qkT = [act.tile([128, 2 * N], FP8, tag=f"qkT{b}") for b in range(12)]
                for b in range(12):
                    ps = ps_mm.tile([128, TT], FP32, tag="ps_mm")
                    for g in range(KG):
                        nc.tensor.matmul(
                            ps[:, :], wqk[g][:, :, b * 128:(b + 1) * 128],
                            xnT[g][:, :, :], start=(g == 0), stop=(g == KG - 1),
                            perf_mode=DR)
                    nc.vector.tensor_scalar(
                        qkT[b][:, :], ps[:, :], SQ / SW, qkb[:, b:b + 1],
                        op0=AL.mult, op1=AL.add)

                # ---------------- v (act-stationary DR) + vb row ----------------
                vt = {}
                for e in range(2):
                    v8 = act.tile([128, 2, H, 68], FP8, tag=f"vt{e}")
                    nc.gpsimd.memset(v8[:, :, :, :], 0.0)
                    nc.gpsimd.memset(v8[:, :, :, D:D + 1], SV)
                    vt[e] = v8
                for e in range(2):
                    for j, (toff, tcnt) in enumerate(T_TILES):
                        tp = T_PADS[j]
                        ts = e * EN + toff
                        for ci, (coff, csz) in enumerate(C_CHUNKS):
                            nh = csz // D
                            h0 = coff // D
                            ps = ps_mm.tile([128, 8, D], FP32, tag="ps_mm")
                            nc.tensor.matmul(
                                ps[:tp, :nh, :],
                                ones_x[:, :, :tp],
                                wvb[:, :, coff:coff + csz],
                                start=True, stop=False, perf_mode=DR)
                            for g in range(KG):
                                nc.tensor.matmul(
                                    ps[:tp, :nh, :],
                                    xnT[g][:, :, ts:ts + tp],
                                    wv[g][:, :, coff:coff + csz],
                                    start=False, stop=(g == KG - 1), perf_mode=DR)
                            nc.vector.tensor_scalar(
                                vt[e][:tcnt, j, h0:h0 + nh, 0:D],
                                ps[:tcnt, :nh, :], SV / SW, None, op0=AL.mult)

                # ---------------- attention ----------------
                aT = [act.tile([128, 2, 2 * N], FP8, tag=f"aT{g}") for g in range(KG)]
                for e in range(2):
                    for h in range(H):
                        qrow = 64 * (h % 2)
                        qt = qkT[h // 2]
                        kt = qkT[6 + h // 2]
                        et = expp.tile([128, 2, EN], FP8, tag="et")
                        nc.gpsimd.memset(et[64:, 1, :], 0.0)
                        nc.gpsimd.memset(et[:, :, N:], 0.0)
                        for j2, (tkoff, tkcnt) in enumerate(T_TILES):
                            tkp = T_PADS[j2]
                            L = ps_at.tile([128, EN], FP32, tag="ps_at")
                            nc.tensor.matmul(
                                L[:tkp, :EN],
                                kt[qrow:qrow + 64, e * EN + tkoff: e * EN + tkoff + tkp],
                                qt[qrow:qrow + 64, e * EN: e * EN + EN],
                                start=True, stop=False)
                            nc.tensor.matmul(
                                L[:tkp, :EN],
                                ident8[:tkp, :tkp],
                                rpb8[j2][:tkp, h, :],
                                start=False, stop=True)
                            nc.scalar.activation(et[:tkcnt, j2, :N], L[:tkcnt, :N],
                                                 AF.Exp, scale=1.0 / (SQ * SQ))
                        O = ps_at.tile([68, EN], FP32, tag="ps_at")
                        nc.tensor.matmul(
                            O[:68, :EN],
                            vt[e][:, :, h, :],
                            et[:, :, :], perf_mode=DR)
                        lden = small.tile([1, N], FP32, tag="lden")
                        rb = small.tile([1, N], BF16, tag="recip")
                        nc.scalar.activation(lden[:, :], O[D:D + 1, :N], AF.Ln)
                        nc.scalar.activation(rb[:, :], lden[:, :], AF.Exp, scale=-1.0)
                        Dn = ps_at.tile([64, N], FP32, tag="ps_at")
                        nc.tensor.matmul(Dn[:, :], ones_bf[0:1, :], rb[0:1, :])
                        Dsb = expp.tile([64, N], BF16, tag="Dsb")
                        nc.scalar.copy(Dsb[:, :], Dn[:, :])
                        nc.vector.tensor_tensor(
                            aT[h // 4][64 * (h % 2):64 * (h % 2) + 64, (h % 4) // 2,
                                       e * EN: e * EN + N],
                            O[0:D, :N], Dsb[:, :], op=AL.mult)

                # ---------------- proj + residual -> x1 ----------------
                x1 = {}
                for e in range(2):
                    for j, (toff, tcnt) in enumerate(T_TILES):
                        tp = T_PADS[j]
                        xt = resid.tile([128, C], FP32, tag=f"x1_{e}{j}")
                        ts = e * EN + toff
                        for ci, (coff, csz) in enumerate(C_CHUNKS):
                            ps = ps_mm.tile([128, 512], FP32, tag="ps_mm")
                            for g in range(KG):
                                nc.tensor.matmul(
                                    ps[:tp, :csz],
                                    aT[g][:, :, ts:ts + tp],
                                    wp[g][:, :, coff:coff + csz],
                                    start=(g == 0), stop=(g == KG - 1), perf_mode=DR)
                            nc.vector.tensor_tensor(
                                xt[:tcnt, coff:coff + csz],
                                ps[:tcnt, :csz],
                                x0[(e, j)][:tcnt, coff:coff + csz], op=AL.add)
                        x1[(e, j)] = xt

                # ---------------- LN2 + fp8 transpose ----------------
                hnT = ln_transpose(x1, "ln2")

                # ---------------- fc1 + gelu -> hT ----------------
                hT = [act.tile([128, 2, 2 * N], FP8, tag=f"hT{g}") for g in range(KG2)]
                for ob in range(24):
                    ps = ps_mm.tile([128, TT], FP32, tag="ps_mm")
                    for g in range(KG):
                        nc.tensor.matmul(
                            ps[:, :], wf1[g][:, :, ob * 128:(ob + 1) * 128],
                            hnT[g][:, :, :], start=(g == 0), stop=(g == KG - 1),
                            perf_mode=DR)
                    nc.scalar.activation(
                        hT[ob // 2][:, ob % 2, :], ps[:, :], AF.Gelu,
                        scale=1.0 / SW, bias=f1b[:, ob:ob + 1])

                # ---------------- fc2 + residual -> y ----------------
                for e in range(2):
                    bidx = 2 * s + e
                    for j, (toff, tcnt) in enumerate(T_TILES):
                        tp = T_PADS[j]
                        ot = resid.tile([128, C], FP32, tag=f"x0_{e}{j}", name=f"out_{e}{j}")
                        ts = e * EN + toff
                        for ci, (coff, csz) in enumerate(C_CHUNKS):
                            ps = ps_mm.tile([128, 512], FP32, tag="ps_mm")
                            for g in range(KG2):
                                nc.tensor.matmul(
                                    ps[:tp, :csz],
                                    hT[g][:, :, ts:ts + tp],
                                    wf2[g][:, :, coff:coff + csz],
                                    start=(g == 0), stop=(g == KG2 - 1), perf_mode=DR)
                            nc.vector.tensor_tensor(
                                ot[:tcnt, coff:coff + csz],
                                ps[:tcnt, :csz],
                                x1[(e, j)][:tcnt, coff:coff + csz], op=AL.add)
                        nc.gpsimd.dma_start(y_d[bidx, toff:toff + tcnt, :], ot[:tcnt, :])

    return nc


def fold_weights(inputs):
    """Host-side folding into fp8 DoubleRow layouts. Returns per-core dict."""
    import ml_dtypes
    f8 = ml_dtypes.float8_e4m3
    bf = ml_dtypes.bfloat16
    f32 = np.float32
    g = {k: np.asarray(v) for k, v in inputs.items()}
    n1w, n1b = g["n1_w"].astype(f32), g["n1_b"].astype(f32)
    n2w, n2b = g["n2_w"].astype(f32), g["n2_b"].astype(f32)
    g1, g2 = g["gamma1"].astype(f32), g["gamma2"].astype(f32)
    qkv_w = g["qkv_w"].astype(f32)
    q_bias, v_bias = g["q_bias"].astype(f32), g["v_bias"].astype(f32)
    proj_w, proj_b = g["proj_w"].astype(f32), g["proj_b"].astype(f32)
    fc1_w, fc1_b = g["fc1_w"].astype(f32), g["fc1_b"].astype(f32)
    fc2_w, fc2_b = g["fc2_w"].astype(f32), g["fc2_b"].astype(f32)
    assert np.all(proj_b == 0), "kernel assumes proj_b == 0"

    qkv_bias = np.concatenate([q_bias, np.zeros_like(q_bias), v_bias])
    Wq = qkv_w * n1w[None, :]                       # LN affine fold
    bq = qkv_bias + qkv_w @ n1b
    scale = D ** -0.5
    Wq[:C] *= scale
    bq[:C] *= scale

    def dr_pack(wT, ngroups):
        # wT: [in_features, out] -> [ngroups, 128, 2, out]
        nin = wT.shape[0]
        assert nin == ngroups * 256
        return np.ascontiguousarray(
            wT.reshape(ngroups, 2, 128, -1).transpose(0, 2, 1, 3))

    wqk = dr_pack((SW * Wq[:2 * C].T), KG).astype(f8)
    wv = dr_pack((SW * Wq[2 * C:].T), KG).astype(f8)
    wvb = np.zeros((1, 2, C), np.float32)
    wvb[0, 0, :] = SW * bq[2 * C:]
    Pw = (g1[:, None] * proj_w)
    wp = dr_pack((R * Pw.T), KG).astype(f8)
    F1 = fc1_w * n2w[None, :]
    f1b_full = fc1_b + fc1_w @ n2b
    wf1 = dr_pack((SW * F1.T), KG).astype(f8)
    F2 = g2[:, None] * fc2_w
    wf2 = dr_pack((R * F2.T), KG2).astype(f8)

    qkb = (SQ * bq[:2 * C]).reshape(12, 128).T.copy()
    f1b = f1b_full.reshape(24, 128).T.copy()

    table = g["rel_bias_table"].astype(f32)
    idx = np.asarray(g["rel_index"]).reshape(-1)
    rpb = table[idx].reshape(N, N, H).transpose(2, 0, 1)   # [h, tq, tk]
    rpbT = rpb.transpose(0, 2, 1)                          # [h, tk, tq]
    rpb0 = np.zeros((128, H, 200), np.float32)
    rpb1 = np.zeros((72, H, 200), np.float32)
    rpb0[:, :, :N] = (SQ * SQ) * rpbT[:, :128, :].transpose(1, 0, 2)
    rpb1[:69, :, :N] = (SQ * SQ) * rpbT[:, 128:, :].transpose(1, 0, 2)

    return {
        "wqk": wqk, "wv": wv, "wvb": wvb.astype(f8), "wp": wp,
        "wf1": wf1, "wf2": wf2,
        "qkb": np.ascontiguousarray(qkb), "f1b": np.ascontiguousarray(f1b),
        "rpb0": rpb0.astype(f8), "rpb1": rpb1.astype(f8),
    }, (g2 * fc2_b).astype(f32)


_CACHE = {}


def _get_nc():
    if "nc" not in _CACHE:
        nc = build_nc()
        patched = _legalize_waits(nc.to_json_bytes())
        nc.to_json_bytes = lambda: patched
        _CACHE["nc"] = nc
    return _CACHE["nc"]


def kernel(**inputs):
    from concourse.bass_utils import run_bass_kernel_spmd
    nc = _get_nc()
    folded, f2b_host = fold_weights(inputs)
    x = np.ascontiguousarray(np.asarray(inputs["x"], dtype=np.float32))
    assert x.shape == (B, N, C), x.shape
    xs = R * x
    in_maps = []
    for c in range(NCORES):
        m = dict(folded)
        m["x"] = np.ascontiguousarray(xs[c * BPC:(c + 1) * BPC])
        in_maps.append(m)
    res = run_bass_kernel_spmd(nc, in_maps, core_ids=list(range(NCORES)))
    out = np.concatenate([res.results[c]["y"] for c in range(NCORES)], axis=0)
    return (out * (1.0 / R) + f2b_host).astype(np.float32)
